# revision 2
# baseline (speedup 1.0000x reference)
"""PredRNN (ConvLSTM enc -> BN -> ConvLSTM dec -> BN -> Conv3D -> sigmoid) on 8 trn2 cores.

Sharding: data-parallel over batch (B=8), one sample per core. Per core:
channel-partition layout, 3x3 convs as 9 shift-window matmuls from padded
bf16 image buffers, gates evicted from PSUM with fused hard-sigmoid, cell
state bf16 in SBUF, decoder K=128 fused [e|h_dec] matmuls, Conv3D via
frame-pair K=128 matmuls with host-baked lhsT, sigmoid eviction.
"""
import sys

sys.path.insert(0, "/opt/trn_rl_repo")
import numpy as np
import ml_dtypes

import concourse.bass as bass
import concourse.tile as tile
from concourse import mybir
from concourse.vector_clock import ScopedClock

BF16 = mybir.dt.bfloat16
F32 = mybir.dt.float32
AF = mybir.ActivationFunctionType
ALU = mybir.AluOpType

T, H, W, F, C = 10, 128, 128, 64, 3
PW = H + 2
NBLK = H // 4
NPIX = H * W
BN_EPS = 1e-3
BFP = ml_dtypes.bfloat16


def _patched_drain_and_barrier(self, tick_clock, wait_clock):
    nc = self.nc
    carrier = nc.sync.nop(nofuse=True, hint="drain_waits")
    wait_clock.add_sem_waits(carrier.ins, ScopedClock({None: tick_clock.global_clock}))
    si = carrier.ins.sync_info
    waits = list(si.on_wait) if si is not None else []
    if len(waits) > 1:
        si.on_wait = waits[:1]
        for w in waits[1:]:
            n = nc.sync.nop(nofuse=True, hint="drain_waits")
            n.ins.sync_info = mybir.SyncInfo(on_wait=[w], on_update=[])
    nc.sync.drain()
    nc.all_engine_barrier()
    popped = nc._tile_sem_poison_stack.pop()
    assert popped is self._sem_poison
    nc.clear_and_free_semaphores(list(self.sems.allocated().values()))
    nc.all_engine_barrier()


tile.TileContext._drain_and_barrier = _patched_drain_and_barrier


def split_multi_waits(nc, max_keep=1):
    """Walrus codegen rejects >1 sem wait on compute instructions; hoist
    extras onto same-engine single-wait NOPs inserted just before."""
    n_split = 0
    for fn in nc.m.functions:
        for blk in fn.blocks:
            insts = blk.instructions
            i = 0
            while i < len(insts):
                inst = insts[i]
                si = inst.sync_info
                waits = list(si.on_wait) if si is not None and si.on_wait else []
                if len(waits) > max_keep:
                    for j, w in enumerate(waits[:-max_keep]):
                        nop = mybir.InstNoOp(
                            name=f"{inst.name}_w{j}",
                            engine=inst.engine,
                            sync_info=mybir.SyncInfo(on_wait=[w], on_update=[]),
                            bass_nofuse=True,
                            ins=[],
                            outs=[],
                        )
                        insts.insert(i, nop)
                        i += 1
                    si.on_wait = waits[-max_keep:]
                    n_split += 1
                i += 1
    return n_split


def _build(do_split=True):
    nc = bass.Bass()
    xim_d = nc.dram_tensor("xim", [T, 27, NPIX], BF16, kind="ExternalInput")
    encw_d = nc.dram_tensor("encw", [128, 9 * 256], BF16, kind="ExternalInput")
    encxw_d = nc.dram_tensor("encxw", [27, 256], BF16, kind="ExternalInput")
    decw_d = nc.dram_tensor("decw", [128, 9 * 256], BF16, kind="ExternalInput")
    w3_d = nc.dram_tensor("w3", [128, 45 * 30], BF16, kind="ExternalInput")
    aps_d = nc.dram_tensor("aps", [128, 8], F32, kind="ExternalInput")
    bout_d = nc.dram_tensor("bout", [30, 1], F32, kind="ExternalInput")
    dstage = nc.dram_tensor("dstage", [T, F, PW, PW], BF16, kind="Internal")
    y_d = nc.dram_tensor("y", [30, H, W], F32, kind="ExternalOutput")

    with tile.TileContext(nc) as tc:
        with tc.tile_pool(name="wp", bufs=1) as wp:
            encw = wp.tile([128, 9, 256], BF16)
            encxw = wp.tile([27, 256], BF16)
            decw = wp.tile([128, 9, 256], BF16)
            w3t = wp.tile([128, 45, 30], BF16)
            aps = wp.tile([128, 8], F32)
            bout = wp.tile([30, 1], F32)
            zb = wp.tile([64, PW], BF16)
            nc.sync.dma_start(encw[:], encw_d[:].rearrange("p (s g) -> p s g", s=9))
            nc.sync.dma_start(encxw[:], encxw_d[:])
            nc.sync.dma_start(decw[:], decw_d[:].rearrange("p (s g) -> p s g", s=9))
            nc.sync.dma_start(w3t[:], w3_d[:].rearrange("p (s g) -> p s g", s=45))
            nc.sync.dma_start(aps[:], aps_d[:])
            nc.sync.dma_start(bout[:], bout_d[:])
            nc.vector.memset(zb[:], 0.0)

            with tc.tile_pool(name="sp", bufs=1) as sp, \
                 tc.tile_pool(name="tp", bufs=1) as tp, \
                 tc.tile_pool(name="pp", bufs=1, space=bass.MemorySpace.PSUM) as pp:
                E = sp.tile([128, PW, PW], BF16)
                B0 = sp.tile([128, PW, PW], BF16)
                B1 = sp.tile([128, PW, PW], BF16)
                ct = sp.tile([128, NPIX], BF16)
                nc.vector.memset(E[:], 0.0)
                nc.vector.memset(B0[:], 0.0)
                nc.vector.memset(B1[:], 0.0)
                nc.vector.memset(ct[:], 0.0)
                for t in range(T):
                    nc.gpsimd.dma_start(dstage[t, :, 0, :], zb[:])
                    nc.gpsimd.dma_start(dstage[t, :, PW - 1, :], zb[:])
                    nc.gpsimd.dma_start(dstage[t, :, :, 0], zb[:])
                    nc.gpsimd.dma_start(dstage[t, :, :, PW - 1], zb[:])
                Bs = [B0, B1]

                def tmp(tag):
                    return tp.tile([128, 512], F32, tag=tag, bufs=2, name=tag)

                for t in range(T):
                    pt, pv = 64 * (t % 2), 64 * ((t - 1) % 2)
                    Bcur, Bnext = Bs[t % 2], Bs[(t + 1) % 2]
                    # ---------------- encoder ----------------
                    for b in range(NBLK):
                        r0, c0 = 4 * b, 512 * b
                        xs = tp.tile([27, 512], BF16, tag="xs", bufs=2)
                        nc.sync.dma_start(xs[:], xim_d[t, :, c0:c0 + 512])
                        pA = pp.tile([128, 512], F32, tag="pA", bufs=2)
                        pB = pp.tile([128, 512], F32, tag="pB", bufs=2)
                        for ch, ps in ((0, pA), (1, pB)):
                            for s in range(9):
                                ky, kx = s // 3, s % 3
                                nc.tensor.matmul(
                                    ps[:],
                                    encw[pv:pv + 64, s, 128 * ch:128 * ch + 128],
                                    E[pv:pv + 64, r0 + ky:r0 + ky + 4, kx:kx + 128],
                                    start=(s == 0), stop=False)
                            nc.tensor.matmul(
                                ps[:], encxw[:, 128 * ch:128 * ch + 128], xs[:],
                                start=False, stop=True)
                        t_if, t_tc, t_o = tmp("t_if"), tmp("t_tc"), tmp("t_o")
                        t_s, t_s2, t_th, t_h = tmp("t_s"), tmp("t_s2"), tmp("t_th"), tmp("t_h")
                        cblk = ct[64:128, c0:c0 + 512]
                        nc.scalar.activation(t_if[:], pA[:], AF.Relu, bias=aps[:, 0:1], scale=0.2)
                        nc.scalar.activation(t_tc[0:64, :], pB[0:64, :], AF.Tanh, bias=aps[0:64, 2:3], scale=1.0)
                        nc.scalar.activation(t_o[0:64, :], pB[64:128, :], AF.Relu, bias=aps[64:128, 2:3], scale=0.2)
                        nc.vector.scalar_tensor_tensor(t_s[64:128, :], t_if[0:64, :], 1.0, t_tc[0:64, :], ALU.min, ALU.mult)
                        nc.vector.scalar_tensor_tensor(t_s2[64:128, :], t_if[64:128, :], 1.0, cblk, ALU.min, ALU.mult)
                        nc.vector.tensor_tensor(cblk, t_s[64:128, :], t_s2[64:128, :], ALU.add)
                        nc.scalar.activation(t_th[0:64, :], cblk, AF.Tanh)
                        nc.vector.scalar_tensor_tensor(t_h[0:64, :], t_o[0:64, :], 1.0, t_th[0:64, :], ALU.min, ALU.mult)
                        hr = t_h[0:64, :].rearrange("p (a b) -> p a b", a=4)
                        nc.gpsimd.tensor_copy(E[pt:pt + 64, r0 + 1:r0 + 5, 1:1 + W], hr)
                        nc.vector.tensor_scalar(
                            Bcur[0:64, r0 + 1:r0 + 5, 1:1 + W], hr,
                            aps[0:64, 4:5], aps[0:64, 5:6], ALU.mult, ALU.add)
                    # ---------------- decoder ----------------
                    for b in range(NBLK):
                        r0, c0 = 4 * b, 512 * b
                        pA = pp.tile([128, 512], F32, tag="pA", bufs=2)
                        pB = pp.tile([128, 512], F32, tag="pB", bufs=2)
                        for ch, ps in ((0, pA), (1, pB)):
                            for s in range(9):
                                ky, kx = s // 3, s % 3
                                nc.tensor.matmul(
                                    ps[:],
                                    decw[:, s, 128 * ch:128 * ch + 128],
                                    Bcur[:, r0 + ky:r0 + ky + 4, kx:kx + 128],
                                    start=(s == 0), stop=(s == 8))
                        t_if, t_tc, t_o = tmp("t_if"), tmp("t_tc"), tmp("t_o")
                        t_s, t_s2, t_th, t_h = tmp("t_s"), tmp("t_s2"), tmp("t_th"), tmp("t_h")
                        cblk = ct[0:64, c0:c0 + 512]
                        # chunk A is [f|i] (host-permuted columns)
                        nc.scalar.activation(t_if[:], pA[:], AF.Relu, bias=aps[:, 1:2], scale=0.2)
                        nc.scalar.activation(t_tc[64:128, :], pB[0:64, :], AF.Tanh, bias=aps[0:64, 3:4], scale=1.0)
                        nc.scalar.activation(t_o[0:64, :], pB[64:128, :], AF.Relu, bias=aps[64:128, 3:4], scale=0.2)
                        nc.vector.scalar_tensor_tensor(t_s2[0:64, :], t_if[0:64, :], 1.0, cblk, ALU.min, ALU.mult)
                        nc.vector.scalar_tensor_tensor(t_s[0:64, :], t_if[64:128, :], 1.0, t_tc[64:128, :], ALU.min, ALU.mult)
                        nc.vector.tensor_tensor(cblk, t_s[0:64, :], t_s2[0:64, :], ALU.add)
                        nc.scalar.activation(t_th[0:64, :], cblk, AF.Tanh)
                        nc.vector.scalar_tensor_tensor(t_h[0:64, :], t_o[0:64, :], 1.0, t_th[0:64, :], ALU.min, ALU.mult)
                        hr = t_h[0:64, :].rearrange("p (a b) -> p a b", a=4)
                        nc.gpsimd.tensor_copy(Bnext[64:128, r0 + 1:r0 + 5, 1:1 + W], hr)
                        dtmp = tp.tile([64, 512], BF16, tag="dtmp", bufs=2)
                        nc.vector.tensor_scalar(
                            dtmp[:], t_h[0:64, :],
                            aps[0:64, 6:7], aps[0:64, 7:8], ALU.mult, ALU.add)
                        nc.gpsimd.dma_start(
                            dstage[t, :, r0 + 1:r0 + 5, 1:1 + W],
                            dtmp[:].rearrange("p (a b) -> p a b", a=4))

            # ---------------- conv3d + sigmoid ----------------
            with tc.tile_pool(name="cp", bufs=1) as cp, \
                 tc.tile_pool(name="cpp", bufs=1, space=bass.MemorySpace.PSUM) as cpp:
                D = [cp.tile([128, PW, PW], BF16, name=f"D{q}") for q in range(5)]
                for q in range(5):
                    nc.gpsimd.dma_start(D[q][0:64, :, :], dstage[2 * q, :, :, :])
                    nc.gpsimd.dma_start(D[q][64:128, :, :], dstage[2 * q + 1, :, :, :])
                for b in range(NBLK):
                    r0 = 4 * b
                    py = cpp.tile([30, 512], F32, tag="py", bufs=2)
                    k = 0
                    for q in range(5):
                        for s in range(9):
                            ky, kx = s // 3, s % 3
                            nc.tensor.matmul(
                                py[:], w3t[:, q * 9 + s, :],
                                D[q][:, r0 + ky:r0 + ky + 4, kx:kx + 128],
                                start=(k == 0), stop=(k == 44))
                            k += 1
                    ty = cp.tile([30, 512], F32, tag="ty", bufs=2)
                    nc.scalar.activation(ty[:], py[:], AF.Sigmoid, bias=bout[:], scale=1.0)
                    nc.scalar.dma_start(
                        y_d[:, r0:r0 + 4, :],
                        ty[:].rearrange("p (a b) -> p a b", a=4))

    if do_split:
        split_multi_waits(nc)
    nc.finalize()
    return nc


def _prep(inputs):
    x = np.asarray(inputs["x"], np.float32)
    xpad = np.zeros((8, T, PW, PW, C), np.float32)
    xpad[:, :, 1:1 + H, 1:1 + W, :] = x
    xim = np.empty((8, T, 27, NPIX), BFP)
    for ky in range(3):
        for kx in range(3):
            s = ky * 3 + kx
            v = xpad[:, :, ky:ky + H, kx:kx + W, :]
            xim[:, :, s * 3:s * 3 + 3, :] = (
                v.transpose(0, 1, 4, 2, 3).reshape(8, T, 3, NPIX).astype(BFP))

    enc_Wh = np.asarray(inputs["enc_Wh"], np.float32)
    enc_Wx = np.asarray(inputs["enc_Wx"], np.float32)
    dec_Wx = np.asarray(inputs["dec_Wx"], np.float32)
    dec_Wh = np.asarray(inputs["dec_Wh"], np.float32)
    out_W = np.asarray(inputs["out_W"], np.float32)

    encw = np.zeros((128, 9, 256), np.float32)
    decw = np.zeros((128, 9, 256), np.float32)
    perm = np.concatenate([np.arange(64, 128), np.arange(0, 64), np.arange(128, 256)])
    for s in range(9):
        ky, kx = s // 3, s % 3
        encw[0:64, s, :] = enc_Wh[ky, kx]
        encw[64:128, s, :] = enc_Wh[ky, kx]
        decw[0:64, s, :] = dec_Wx[ky, kx][:, perm]
        decw[64:128, s, :] = dec_Wh[ky, kx][:, perm]
    encxw = enc_Wx.reshape(27, 256)

    w3 = np.zeros((45, 128, 30), np.float32)
    for q in range(5):
        for j in range(2):
            f = 2 * q + j
            for t in range(max(0, f - 1), min(T - 1, f + 1) + 1):
                dt = f - t + 1
                for s in range(9):
                    ky, kx = s // 3, s % 3
                    w3[q * 9 + s, 64 * j:64 * j + 64, 3 * t:3 * t + 3] = out_W[dt, ky, kx]
    w3 = w3.transpose(1, 0, 2)  # [128, 45, 30]

    enc_b = np.asarray(inputs["enc_b"], np.float32)
    dec_b = np.asarray(inputs["dec_b"], np.float32)
    s_e = np.asarray(inputs["enc_gamma"], np.float32) / np.sqrt(
        np.asarray(inputs["enc_var"], np.float32) + BN_EPS)
    t_e = np.asarray(inputs["enc_beta"], np.float32) - np.asarray(inputs["enc_mean"], np.float32) * s_e
    s_d = np.asarray(inputs["dec_gamma"], np.float32) / np.sqrt(
        np.asarray(inputs["dec_var"], np.float32) + BN_EPS)
    t_d = np.asarray(inputs["dec_beta"], np.float32) - np.asarray(inputs["dec_mean"], np.float32) * s_d

    aps = np.zeros((128, 8), np.float32)
    aps[0:64, 0] = 0.2 * enc_b[0:64] + 0.5          # enc i
    aps[64:128, 0] = 0.2 * enc_b[64:128] + 0.5      # enc f
    aps[0:64, 1] = 0.2 * dec_b[64:128] + 0.5        # dec f (chunk A is [f|i])
    aps[64:128, 1] = 0.2 * dec_b[0:64] + 0.5        # dec i
    aps[0:64, 2] = enc_b[128:192]                   # enc c~ (tanh bias)
    aps[64:128, 2] = 0.2 * enc_b[192:256] + 0.5     # enc o
    aps[0:64, 3] = dec_b[128:192]
    aps[64:128, 3] = 0.2 * dec_b[192:256] + 0.5
    aps[0:64, 4] = s_e
    aps[0:64, 5] = t_e
    aps[0:64, 6] = s_d
    aps[0:64, 7] = t_d
    bout = np.tile(np.asarray(inputs["out_b"], np.float32), T).reshape(30, 1)

    shared = {
        "encw": encw.reshape(128, 9 * 256).astype(BFP),
        "encxw": encxw.astype(BFP),
        "decw": decw.reshape(128, 9 * 256).astype(BFP),
        "w3": w3.reshape(128, 45 * 30).astype(BFP),
        "aps": aps,
        "bout": bout,
    }
    return [dict(shared, xim=np.ascontiguousarray(xim[c])) for c in range(8)]


_CACHE = {}


def kernel(**inputs):
    if "nc" not in _CACHE:
        _CACHE["nc"] = _build()
    nc = _CACHE["nc"]
    in_maps = _prep(inputs)
    from concourse.bass_utils import run_bass_kernel_spmd
    res = run_bass_kernel_spmd(nc, in_maps, core_ids=list(range(8)))
    kernel.last_exec_ns = res.exec_time_ns
    kernel.last_res = res
    y = np.stack([
        np.asarray(res.results[c]["y"], np.float32)
        .reshape(T, 3, H, W).transpose(0, 2, 3, 1)
        for c in range(8)
    ])
    return y



# revision 7
# speedup vs baseline: 1.7072x; 1.7072x over previous
"""PredRNN (ConvLSTM enc -> BN -> ConvLSTM dec -> BN -> Conv3D -> sigmoid) on 8 trn2 cores.

Sharding: data-parallel over batch (B=8), one sample per core. Per core:
- Both BNs folded into downstream conv weights; pad-ring constants fix the
  zero-padded borders, so no BN compute on device.
- Encoder hidden conv: h stored twice in the partition dim (plain + shifted
  one column left), so 2 of the 9 taps fuse into one K=128 matmul; all
  encoder matmuls are K=128 (zero-padded weights) to keep the PE HAM clock
  warm (K<128 rhs measured at half clock).
- Decoder: K=128 [h_enc | h_dec] stacked, 9 taps x 2 gate-chunks.
- N=1024 superblocks (8 image rows, 2 PSUM banks per gate-chunk) to amortize
  ACT/DVE fixed overheads; gates evicted with fused hard-sigmoid.
- Conv3D via frame-pair K=128 matmuls with host-baked lhsT, sigmoid eviction.
"""
import sys

sys.path.insert(0, "/opt/trn_rl_repo")
import numpy as np
import ml_dtypes

import concourse.bass as bass
import concourse.tile as tile
from concourse import mybir
from concourse.vector_clock import ScopedClock

BF16 = mybir.dt.bfloat16
F32 = mybir.dt.float32
AF = mybir.ActivationFunctionType
ALU = mybir.AluOpType

T, H, W, F, C = 10, 128, 128, 64, 3
PW = H + 2
NSB = H // 8          # 16 superblocks of 8 rows / 1024 px
NPIX = H * W
BN_EPS = 1e-3
BFP = ml_dtypes.bfloat16


def _patched_drain_and_barrier(self, tick_clock, wait_clock):
    nc = self.nc
    carrier = nc.sync.nop(nofuse=True, hint="drain_waits")
    wait_clock.add_sem_waits(carrier.ins, ScopedClock({None: tick_clock.global_clock}))
    si = carrier.ins.sync_info
    waits = list(si.on_wait) if si is not None else []
    if len(waits) > 1:
        si.on_wait = waits[:1]
        for w in waits[1:]:
            n = nc.sync.nop(nofuse=True, hint="drain_waits")
            n.ins.sync_info = mybir.SyncInfo(on_wait=[w], on_update=[])
    nc.sync.drain()
    nc.all_engine_barrier()
    popped = nc._tile_sem_poison_stack.pop()
    assert popped is self._sem_poison
    nc.clear_and_free_semaphores(list(self.sems.allocated().values()))
    nc.all_engine_barrier()


tile.TileContext._drain_and_barrier = _patched_drain_and_barrier


def split_multi_waits(nc, max_keep=1):
    """Walrus codegen rejects >1 sem wait on compute instructions; hoist
    extras onto same-engine single-wait NOPs inserted just before."""
    n_split = 0
    for fn in nc.m.functions:
        for blk in fn.blocks:
            insts = blk.instructions
            i = 0
            while i < len(insts):
                inst = insts[i]
                si = inst.sync_info
                waits = list(si.on_wait) if si is not None and si.on_wait else []
                if len(waits) > max_keep:
                    for j, w in enumerate(waits[:-max_keep]):
                        nop = mybir.InstNoOp(
                            name=f"{inst.name}_w{j}",
                            engine=inst.engine,
                            sync_info=mybir.SyncInfo(on_wait=[w], on_update=[]),
                            bass_nofuse=True,
                            ins=[],
                            outs=[],
                        )
                        insts.insert(i, nop)
                        i += 1
                    si.on_wait = waits[-max_keep:]
                    n_split += 1
                i += 1
    return n_split


def _build(do_split=True):
    nc = bass.Bass()
    xim_d = nc.dram_tensor("xim", [T, 27, NPIX], BF16, kind="ExternalInput")
    encp_d = nc.dram_tensor("encp", [128, 3 * 256], BF16, kind="ExternalInput")
    encs_d = nc.dram_tensor("encs", [128, 3 * 256], BF16, kind="ExternalInput")
    encx_d = nc.dram_tensor("encx", [128, 256], BF16, kind="ExternalInput")
    decw_d = nc.dram_tensor("decw", [128, 9 * 256], BF16, kind="ExternalInput")
    w3_d = nc.dram_tensor("w3", [128, 45 * 30], BF16, kind="ExternalInput")
    aps_d = nc.dram_tensor("aps", [128, 4], F32, kind="ExternalInput")
    bout_d = nc.dram_tensor("bout", [30, 1], F32, kind="ExternalInput")
    ringb_d = nc.dram_tensor("ringb", [128, PW], BF16, kind="ExternalInput")
    ringd_d = nc.dram_tensor("ringd", [64, PW], BF16, kind="ExternalInput")
    dstage = nc.dram_tensor("dstage", [T, F, PW, PW], BF16, kind="Internal")
    y_d = nc.dram_tensor("y", [30, H, W], F32, kind="ExternalOutput")

    with tile.TileContext(nc) as tc:
        with tc.tile_pool(name="wp", bufs=1) as wp:
            encp = wp.tile([128, 3, 256], BF16)
            encs = wp.tile([128, 3, 256], BF16)
            encx = wp.tile([128, 256], BF16)
            decw = wp.tile([128, 9, 256], BF16)
            w3t = wp.tile([128, 45, 30], BF16)
            aps = wp.tile([128, 4], F32)
            bout = wp.tile([30, 1], F32)
            zbB = wp.tile([128, PW], BF16)
            zbD = wp.tile([64, PW], BF16)
            nc.sync.dma_start(encp[:], encp_d[:].rearrange("p (s g) -> p s g", s=3))
            nc.sync.dma_start(encs[:], encs_d[:].rearrange("p (s g) -> p s g", s=3))
            nc.sync.dma_start(encx[:], encx_d[:])
            nc.sync.dma_start(decw[:], decw_d[:].rearrange("p (s g) -> p s g", s=9))
            nc.sync.dma_start(w3t[:], w3_d[:].rearrange("p (s g) -> p s g", s=45))
            nc.sync.dma_start(aps[:], aps_d[:])
            nc.sync.dma_start(bout[:], bout_d[:])
            nc.sync.dma_start(zbB[:], ringb_d[:])
            nc.sync.dma_start(zbD[:], ringd_d[:])

            with tc.tile_pool(name="sp", bufs=1) as sp, \
                 tc.tile_pool(name="tp", bufs=1) as tp, \
                 tc.tile_pool(name="pp", bufs=1, space=bass.MemorySpace.PSUM) as pp:
                E0 = sp.tile([128, PW, PW], BF16)
                E1 = sp.tile([128, PW, PW], BF16)
                B0 = sp.tile([128, PW, PW], BF16)
                B1 = sp.tile([128, PW, PW], BF16)
                CC = sp.tile([128, NPIX], BF16)
                nc.vector.memset(E0[:], 0.0)
                nc.vector.memset(CC[:], 0.0)
                nc.vector.memset(E1[:], 0.0)
                nc.vector.memset(B0[:], 0.0)
                nc.vector.memset(B1[:], 0.0)
                for B in (B0, B1):
                    # top ring = -t_e/s_e (rows 0:64), bottom ring = 0
                    nc.gpsimd.tensor_copy(B[:, 0, :], zbB[:])
                    nc.gpsimd.tensor_copy(B[:, PW - 1, :], zbB[:])
                    nc.gpsimd.tensor_copy(
                        B[:, :, 0:1], zbB[:].rearrange("p (w o) -> p w o", o=1))
                    nc.gpsimd.tensor_copy(
                        B[:, :, PW - 1:PW], zbB[:].rearrange("p (w o) -> p w o", o=1))
                Es = [E0, E1]
                Bs = [B0, B1]

                # init xs staging buffers to zero (rows 27:128 stay zero,
                # multiplied by zero weight rows -> avoids NaN garbage)
                for _ in range(2):
                    xs0 = tp.tile([128, 1024], BF16, tag="xs", bufs=2, name="xs0")
                    nc.vector.memset(xs0[:], 0.0)

                def enc_sb(t, k):
                    Ep, En = Es[t % 2], Es[(t + 1) % 2]
                    Bc = Bs[t % 2]
                    r0, c0 = 8 * k, 1024 * k
                    xs = tp.tile([128, 1024], BF16, tag="xs", bufs=2)
                    nc.sync.dma_start(xs[0:27, :], xim_d[t, :, c0:c0 + 1024])
                    pA = pp.tile([128, 1024], F32, tag="pA", bufs=2)
                    pB = pp.tile([128, 1024], F32, tag="pB", bufs=2)
                    for ch, ps in ((0, pA), (1, pB)):
                        cw = slice(128 * ch, 128 * ch + 128)
                        for hf in range(2):
                            rh = r0 + 4 * hf
                            out = ps[:, 512 * hf:512 * hf + 512]
                            for ky in range(3):  # paired taps (ky,0)+(ky,1)
                                nc.tensor.matmul(
                                    out, encp[:, ky, cw],
                                    Ep[:, rh + ky:rh + ky + 4, 0:128],
                                    start=(ky == 0), stop=False)
                            for ky in range(3):  # single taps (ky,2)
                                nc.tensor.matmul(
                                    out, encs[:, ky, cw],
                                    Ep[:, rh + ky:rh + ky + 4, 2:130],
                                    start=False, stop=False)
                            nc.tensor.matmul(
                                out, encx[:, cw], xs[:, 512 * hf:512 * hf + 512],
                                start=False, stop=True)
                    t_if = tp.tile([128, 1024], F32, tag="t_if", bufs=2, name="t_if")
                    t_u = tp.tile([128, 1024], BF16, tag="t_u", bufs=2, name="t_u")
                    v1 = tp.tile([128, 1024], BF16, tag="t_v", bufs=3, name="v1")
                    v2 = tp.tile([128, 1024], BF16, tag="t_v", bufs=3, name="v2")
                    th = tp.tile([128, 1024], BF16, tag="t_v", bufs=3, name="th")
                    cblk = CC[64:128, c0:c0 + 1024]
                    nc.scalar.activation(t_if[:], pA[:], AF.Relu,
                                         bias=aps[:, 0:1], scale=0.2)
                    nc.scalar.activation(t_u[0:64, :], pB[0:64, :], AF.Tanh,
                                         bias=aps[0:64, 1:2], scale=1.0)
                    nc.scalar.activation(t_u[64:128, :], pB[64:128, :], AF.Relu,
                                         bias=aps[64:128, 1:2], scale=0.2)
                    nc.vector.scalar_tensor_tensor(
                        v1[64:128, :], t_if[0:64, :], 1.0, t_u[0:64, :],
                        ALU.min, ALU.mult)
                    nc.vector.scalar_tensor_tensor(
                        v2[64:128, :], t_if[64:128, :], 1.0, cblk,
                        ALU.min, ALU.mult)
                    nc.vector.tensor_tensor(cblk, v1[64:128, :], v2[64:128, :],
                                            ALU.add)
                    nc.scalar.activation(th[64:128, :], cblk, AF.Tanh)
                    # h -> E_next copy1 (strided image write, fused o*tanh(c))
                    nc.vector.scalar_tensor_tensor(
                        En[0:64, r0 + 1:r0 + 9, 1:1 + W],
                        t_u[64:128, :].rearrange("p (a b) -> p a b", a=8), 1.0,
                        th[64:128, :].rearrange("p (a b) -> p a b", a=8),
                        ALU.min, ALU.mult)
                    # copy2: shifted one column left (for paired taps)
                    nc.gpsimd.tensor_copy(
                        En[64:128, r0 + 1:r0 + 9, 0:W],
                        En[0:64, r0 + 1:r0 + 9, 1:1 + W])
                    # B_cur top: h_enc for the decoder (SBUF->SBUF DMA)
                    nc.sync.dma_start(
                        Bc[0:64, r0 + 1:r0 + 9, 1:1 + W],
                        En[0:64, r0 + 1:r0 + 9, 1:1 + W])

                def dec_sb(t, k):
                    Bc, Bn = Bs[t % 2], Bs[(t + 1) % 2]
                    r0, c0 = 8 * k, 1024 * k
                    pA = pp.tile([128, 1024], F32, tag="pA", bufs=2)
                    pB = pp.tile([128, 1024], F32, tag="pB", bufs=2)
                    for ch, ps in ((0, pA), (1, pB)):
                        cw = slice(128 * ch, 128 * ch + 128)
                        for hf in range(2):
                            rh = r0 + 4 * hf
                            out = ps[:, 512 * hf:512 * hf + 512]
                            for s in range(9):
                                ky, kx = s // 3, s % 3
                                nc.tensor.matmul(
                                    out, decw[:, s, cw],
                                    Bc[:, rh + ky:rh + ky + 4, kx:kx + 128],
                                    start=(s == 0), stop=(s == 8))
                    t_if = tp.tile([128, 1024], F32, tag="t_if", bufs=2, name="t_if")
                    t_u = tp.tile([128, 1024], BF16, tag="t_u", bufs=2, name="t_u")
                    v1 = tp.tile([128, 1024], BF16, tag="t_v", bufs=3, name="v1")
                    v2 = tp.tile([128, 1024], BF16, tag="t_v", bufs=3, name="v2")
                    th = tp.tile([128, 1024], BF16, tag="t_v", bufs=3, name="th")
                    cblk = CC[0:64, c0:c0 + 1024]
                    # dec chunk A is [f|i] (host-permuted); tanh/relu ACT
                    # outputs criss-cross partitions so DVE input bases match
                    nc.scalar.activation(t_if[:], pA[:], AF.Relu,
                                         bias=aps[:, 2:3], scale=0.2)
                    nc.scalar.activation(t_u[64:128, :], pB[0:64, :], AF.Tanh,
                                         bias=aps[0:64, 3:4], scale=1.0)
                    nc.scalar.activation(t_u[0:64, :], pB[64:128, :], AF.Relu,
                                         bias=aps[64:128, 3:4], scale=0.2)
                    nc.vector.scalar_tensor_tensor(
                        v1[0:64, :], t_if[64:128, :], 1.0, t_u[64:128, :],
                        ALU.min, ALU.mult)
                    nc.vector.scalar_tensor_tensor(
                        v2[0:64, :], t_if[0:64, :], 1.0, cblk,
                        ALU.min, ALU.mult)
                    nc.vector.tensor_tensor(cblk, v1[0:64, :], v2[0:64, :],
                                            ALU.add)
                    nc.scalar.activation(th[0:64, :], cblk, AF.Tanh)
                    # h_dec -> B_next bottom (strided image write)
                    nc.vector.scalar_tensor_tensor(
                        Bn[64:128, r0 + 1:r0 + 9, 1:1 + W],
                        t_u[0:64, :].rearrange("p (a b) -> p a b", a=8), 1.0,
                        th[0:64, :].rearrange("p (a b) -> p a b", a=8),
                        ALU.min, ALU.mult)
                    # stage h_dec to DRAM for conv3d
                    nc.gpsimd.dma_start(
                        dstage[t, :, r0 + 1:r0 + 9, 1:1 + W],
                        Bn[64:128, r0 + 1:r0 + 9, 1:1 + W])

                for t in range(T):
                    # dstage[t] ring = -t_d/s_d (spatial pad after BN fold)
                    nc.gpsimd.dma_start(dstage[t, :, 0, :], zbD[:])
                    nc.gpsimd.dma_start(dstage[t, :, PW - 1, :], zbD[:])
                    nc.gpsimd.dma_start(
                        dstage[t, :, :, 0:1],
                        zbD[:].rearrange("p (w o) -> p w o", o=1))
                    nc.gpsimd.dma_start(
                        dstage[t, :, :, PW - 1:PW],
                        zbD[:].rearrange("p (w o) -> p w o", o=1))
                    for k in range(NSB):
                        enc_sb(t, k)
                        if k >= 1:
                            dec_sb(t, k - 1)
                    dec_sb(t, NSB - 1)

            # ---------------- conv3d + sigmoid ----------------
            with tc.tile_pool(name="cp", bufs=1) as cp, \
                 tc.tile_pool(name="cpp", bufs=1, space=bass.MemorySpace.PSUM) as cpp:
                D = [cp.tile([128, PW, PW], BF16, name=f"D{q}") for q in range(5)]
                for q in range(5):
                    eng = (nc.sync, nc.gpsimd, nc.scalar)[q % 3]
                    eng.dma_start(D[q][0:64, :, :], dstage[2 * q, :, :, :])
                    eng.dma_start(D[q][64:128, :, :], dstage[2 * q + 1, :, :, :])
                for k in range(NSB):
                    r0 = 8 * k
                    py = cpp.tile([30, 1024], F32, tag="py", bufs=2)
                    for hf in range(2):
                        rh = r0 + 4 * hf
                        out = py[:, 512 * hf:512 * hf + 512]
                        i = 0
                        for q in range(5):
                            for s in range(9):
                                ky, kx = s // 3, s % 3
                                nc.tensor.matmul(
                                    out, w3t[:, q * 9 + s, :],
                                    D[q][:, rh + ky:rh + ky + 4, kx:kx + 128],
                                    start=(i == 0), stop=(i == 44))
                                i += 1
                    ty = cp.tile([30, 1024], F32, tag="ty", bufs=2)
                    nc.scalar.activation(ty[:], py[:], AF.Sigmoid,
                                         bias=bout[:], scale=1.0)
                    nc.scalar.dma_start(
                        y_d[:, r0:r0 + 8, :],
                        ty[:].rearrange("p (a b) -> p a b", a=8))

    if do_split:
        split_multi_waits(nc)
    nc.finalize()
    return nc


def _prep(inputs):
    x = np.asarray(inputs["x"], np.float32)
    xpad = np.zeros((8, T, PW, PW, C), np.float32)
    xpad[:, :, 1:1 + H, 1:1 + W, :] = x
    xim = np.empty((8, T, 27, NPIX), BFP)
    for ky in range(3):
        for kx in range(3):
            s = ky * 3 + kx
            v = xpad[:, :, ky:ky + H, kx:kx + W, :]
            xim[:, :, s * 3:s * 3 + 3, :] = (
                v.transpose(0, 1, 4, 2, 3).reshape(8, T, 3, NPIX).astype(BFP))

    enc_Wh = np.asarray(inputs["enc_Wh"], np.float32)
    enc_Wx = np.asarray(inputs["enc_Wx"], np.float32)
    dec_Wx = np.asarray(inputs["dec_Wx"], np.float32)
    dec_Wh = np.asarray(inputs["dec_Wh"], np.float32)
    out_W = np.asarray(inputs["out_W"], np.float32)
    enc_b = np.asarray(inputs["enc_b"], np.float32)
    dec_b = np.asarray(inputs["dec_b"], np.float32)

    # BN affine folds
    s_e = np.asarray(inputs["enc_gamma"], np.float32) / np.sqrt(
        np.asarray(inputs["enc_var"], np.float32) + BN_EPS)
    t_e = np.asarray(inputs["enc_beta"], np.float32) - np.asarray(
        inputs["enc_mean"], np.float32) * s_e
    s_d = np.asarray(inputs["dec_gamma"], np.float32) / np.sqrt(
        np.asarray(inputs["dec_var"], np.float32) + BN_EPS)
    t_d = np.asarray(inputs["dec_beta"], np.float32) - np.asarray(
        inputs["dec_mean"], np.float32) * s_d

    # encoder hidden-conv weights: paired taps (ky,0)+(ky,1), singles (ky,2)
    encp = np.zeros((128, 3, 256), np.float32)
    encs = np.zeros((128, 3, 256), np.float32)
    for ky in range(3):
        encp[0:64, ky, :] = enc_Wh[ky, 0]
        encp[64:128, ky, :] = enc_Wh[ky, 1]
        encs[0:64, ky, :] = enc_Wh[ky, 2]
    encx = np.zeros((128, 256), np.float32)
    encx[0:27, :] = enc_Wx.reshape(27, 256)

    # decoder weights: rows 0:64 dec_Wx scaled by s_e (enc-BN fold),
    # rows 64:128 dec_Wh; gate chunk A permuted to [f|i] so DVE two-input
    # ops get matching base partitions
    perm = np.concatenate([np.arange(64, 128), np.arange(0, 64),
                           np.arange(128, 256)])
    decw = np.zeros((128, 9, 256), np.float32)
    for s in range(9):
        ky, kx = s // 3, s % 3
        decw[0:64, s, :] = (dec_Wx[ky, kx] * s_e[:, None])[:, perm]
        decw[64:128, s, :] = dec_Wh[ky, kx][:, perm]
    # enc-BN shift folded into decoder bias (interior taps; border taps
    # cancelled by the -t_e/s_e ring in B's top half)
    dec_b_f = dec_b + np.einsum("ykcg,c->g", dec_Wx.reshape(3, 3, 64, 256), t_e)

    # conv3d weights scaled by s_d (dec-BN fold)
    out_W_s = out_W * s_d[None, None, None, :, None]
    w3 = np.zeros((45, 128, 30), np.float32)
    for q in range(5):
        for j in range(2):
            f = 2 * q + j
            for t in range(max(0, f - 1), min(T - 1, f + 1) + 1):
                dt = f - t + 1
                for s in range(9):
                    ky, kx = s // 3, s % 3
                    w3[q * 9 + s, 64 * j:64 * j + 64, 3 * t:3 * t + 3] = \
                        out_W_s[dt, ky, kx]
    w3 = w3.transpose(1, 0, 2)  # [128, 45, 30]

    # conv3d bias: out_b + dec-BN shift over valid temporal taps
    out_b = np.asarray(inputs["out_b"], np.float32)
    bout = np.zeros((30,), np.float32)
    for t in range(T):
        acc = out_b.copy()
        for dt in range(3):
            g = t + dt - 1
            if 0 <= g < T:
                acc = acc + np.einsum("ykcg,c->g", out_W[dt], t_d)
        bout[3 * t:3 * t + 3] = acc
    bout = bout.reshape(30, 1)

    aps = np.zeros((128, 4), np.float32)
    aps[0:64, 0] = 0.2 * enc_b[0:64] + 0.5        # enc i
    aps[64:128, 0] = 0.2 * enc_b[64:128] + 0.5    # enc f
    aps[0:64, 1] = enc_b[128:192]                 # enc c~ (tanh bias)
    aps[64:128, 1] = 0.2 * enc_b[192:256] + 0.5   # enc o
    aps[0:64, 2] = 0.2 * dec_b_f[64:128] + 0.5    # dec f (chunk A is [f|i])
    aps[64:128, 2] = 0.2 * dec_b_f[0:64] + 0.5    # dec i
    aps[0:64, 3] = dec_b_f[128:192]               # dec c~
    aps[64:128, 3] = 0.2 * dec_b_f[192:256] + 0.5 # dec o

    ringb = np.zeros((128, PW), np.float32)
    ringb[0:64, :] = (-t_e / s_e)[:, None]
    ringd = np.tile((-t_d / s_d)[:, None], (1, PW))

    shared = {
        "encp": encp.reshape(128, 3 * 256).astype(BFP),
        "encs": encs.reshape(128, 3 * 256).astype(BFP),
        "encx": encx.astype(BFP),
        "decw": decw.reshape(128, 9 * 256).astype(BFP),
        "w3": w3.reshape(128, 45 * 30).astype(BFP),
        "aps": aps,
        "bout": bout,
        "ringb": ringb.astype(BFP),
        "ringd": ringd.astype(BFP),
    }
    return [dict(shared, xim=np.ascontiguousarray(xim[c])) for c in range(8)]


_CACHE = {}


def kernel(**inputs):
    if "nc" not in _CACHE:
        _CACHE["nc"] = _build()
    nc = _CACHE["nc"]
    in_maps = _prep(inputs)
    from concourse.bass_utils import run_bass_kernel_spmd
    res = run_bass_kernel_spmd(nc, in_maps, core_ids=list(range(8)))
    kernel.last_exec_ns = res.exec_time_ns
    kernel.last_res = res
    y = np.stack([
        np.asarray(res.results[c]["y"], np.float32)
        .reshape(T, 3, H, W).transpose(0, 2, 3, 1)
        for c in range(8)
    ])
    return y


# revision 9
# speedup vs baseline: 1.9936x; 1.1678x over previous
"""PredRNN (ConvLSTM enc -> BN -> ConvLSTM dec -> BN -> Conv3D -> sigmoid) on 8 trn2 cores.

Sharding: data-parallel over batch (B=8), one sample per core. Per core:
- Both BNs folded into downstream conv weights; pad-ring constants fix the
  zero-padded borders, so no BN compute on device.
- Encoder hidden conv: h stored twice in the partition dim (plain + shifted
  one column left), so 2 of the 9 taps fuse into one K=128 matmul; all
  encoder matmuls are K=128 (zero-padded weights) to keep the PE HAM clock
  warm (K<128 rhs measured at half clock).
- Decoder: K=128 [h_enc | h_dec] stacked, 9 taps x 2 gate-chunks.
- N=1024 superblocks (8 image rows, 2 PSUM banks per gate-chunk) to amortize
  ACT/DVE fixed overheads; gates evicted with fused hard-sigmoid.
- Conv3D via frame-pair K=128 matmuls with host-baked lhsT, sigmoid eviction.
"""
import sys

sys.path.insert(0, "/opt/trn_rl_repo")
import numpy as np
import ml_dtypes

import concourse.bass as bass
import concourse.tile as tile
from concourse import mybir
from concourse.vector_clock import ScopedClock

BF16 = mybir.dt.bfloat16
F32 = mybir.dt.float32
AF = mybir.ActivationFunctionType
ALU = mybir.AluOpType

T, H, W, F, C = 10, 128, 128, 64, 3
PW = H + 2
NSB = H // 8          # 16 superblocks of 8 rows / 1024 px
NPIX = H * W
BN_EPS = 1e-3
BFP = ml_dtypes.bfloat16


def _patched_drain_and_barrier(self, tick_clock, wait_clock):
    nc = self.nc
    carrier = nc.sync.nop(nofuse=True, hint="drain_waits")
    wait_clock.add_sem_waits(carrier.ins, ScopedClock({None: tick_clock.global_clock}))
    si = carrier.ins.sync_info
    waits = list(si.on_wait) if si is not None else []
    if len(waits) > 1:
        si.on_wait = waits[:1]
        for w in waits[1:]:
            n = nc.sync.nop(nofuse=True, hint="drain_waits")
            n.ins.sync_info = mybir.SyncInfo(on_wait=[w], on_update=[])
    nc.sync.drain()
    nc.all_engine_barrier()
    popped = nc._tile_sem_poison_stack.pop()
    assert popped is self._sem_poison
    nc.clear_and_free_semaphores(list(self.sems.allocated().values()))
    nc.all_engine_barrier()


tile.TileContext._drain_and_barrier = _patched_drain_and_barrier


def split_multi_waits(nc, max_keep=1):
    """Walrus codegen rejects >1 sem wait on compute instructions; hoist
    extras onto same-engine single-wait NOPs inserted just before."""
    n_split = 0
    for fn in nc.m.functions:
        for blk in fn.blocks:
            insts = blk.instructions
            i = 0
            while i < len(insts):
                inst = insts[i]
                si = inst.sync_info
                waits = list(si.on_wait) if si is not None and si.on_wait else []
                if len(waits) > max_keep:
                    for j, w in enumerate(waits[:-max_keep]):
                        nop = mybir.InstNoOp(
                            name=f"{inst.name}_w{j}",
                            engine=inst.engine,
                            sync_info=mybir.SyncInfo(on_wait=[w], on_update=[]),
                            bass_nofuse=True,
                            ins=[],
                            outs=[],
                        )
                        insts.insert(i, nop)
                        i += 1
                    si.on_wait = waits[-max_keep:]
                    n_split += 1
                i += 1
    return n_split


def _build(do_split=True):
    nc = bass.Bass()
    xim_d = nc.dram_tensor("xim", [T, 27, NPIX], BF16, kind="ExternalInput")
    encp_d = nc.dram_tensor("encp", [128, 3 * 256], BF16, kind="ExternalInput")
    encs_d = nc.dram_tensor("encs", [128, 3 * 256], BF16, kind="ExternalInput")
    encx_d = nc.dram_tensor("encx", [128, 256], BF16, kind="ExternalInput")
    decw_d = nc.dram_tensor("decw", [128, 9 * 256], BF16, kind="ExternalInput")
    w3_d = nc.dram_tensor("w3", [128, 45 * 30], BF16, kind="ExternalInput")
    aps_d = nc.dram_tensor("aps", [128, 4], F32, kind="ExternalInput")
    bout_d = nc.dram_tensor("bout", [30, 1], F32, kind="ExternalInput")
    ringb_d = nc.dram_tensor("ringb", [128, PW], BF16, kind="ExternalInput")
    ringd_d = nc.dram_tensor("ringd", [64, PW], BF16, kind="ExternalInput")
    dstage = nc.dram_tensor("dstage", [T, F, PW, PW], BF16, kind="Internal")
    y_d = nc.dram_tensor("y", [30, H, W], F32, kind="ExternalOutput")

    with tile.TileContext(nc) as tc:
        with tc.tile_pool(name="wp", bufs=1) as wp:
            encp = wp.tile([128, 3, 256], BF16)
            encs = wp.tile([128, 3, 256], BF16)
            encx = wp.tile([128, 256], BF16)
            decw = wp.tile([128, 9, 256], BF16)
            w3t = wp.tile([128, 45, 30], BF16)
            aps = wp.tile([128, 4], F32)
            bout = wp.tile([30, 1], F32)
            zbB = wp.tile([128, PW], BF16)
            zbD = wp.tile([64, PW], BF16)
            nc.sync.dma_start(encp[:], encp_d[:].rearrange("p (s g) -> p s g", s=3))
            nc.sync.dma_start(encs[:], encs_d[:].rearrange("p (s g) -> p s g", s=3))
            nc.sync.dma_start(encx[:], encx_d[:])
            nc.sync.dma_start(decw[:], decw_d[:].rearrange("p (s g) -> p s g", s=9))
            nc.sync.dma_start(w3t[:], w3_d[:].rearrange("p (s g) -> p s g", s=45))
            nc.sync.dma_start(aps[:], aps_d[:])
            nc.sync.dma_start(bout[:], bout_d[:])
            nc.sync.dma_start(zbB[:], ringb_d[:])
            nc.sync.dma_start(zbD[:], ringd_d[:])

            with tc.tile_pool(name="sp", bufs=1) as sp, \
                 tc.tile_pool(name="tp", bufs=1) as tp, \
                 tc.tile_pool(name="pp", bufs=1, space=bass.MemorySpace.PSUM) as pp:
                E0 = sp.tile([128, PW, PW], BF16)
                E1 = sp.tile([128, PW, PW], BF16)
                B0 = sp.tile([128, PW, PW], BF16)
                B1 = sp.tile([128, PW, PW], BF16)
                CC = sp.tile([128, NPIX], BF16)
                # init xs staging buffers to zero first (rows 27:128 stay
                # zero, multiplied by zero weight rows -> avoids NaN garbage)
                for _ in range(2):
                    xs0 = tp.tile([128, 1024], BF16, tag="xs", bufs=2, name="xs0")
                    nc.vector.memset(xs0[:], 0.0)
                nc.vector.memset(E0[:], 0.0)   # h_enc(-1) = 0
                nc.vector.memset(CC[:], 0.0)   # c(-1) = 0
                # E1 only needs its pad ring zeroed (interior fully written
                # at t=0); copy2 tail cols 128:130 also never written
                nc.vector.memset(E1[:, 0, :], 0.0)
                nc.vector.memset(E1[:, PW - 1, :], 0.0)
                nc.vector.memset(E1[:, :, 0:1], 0.0)
                nc.vector.memset(E1[:, :, PW - 2:PW], 0.0)
                # B0 bottom half: h_dec(-1) = 0; B1 interior is written
                # before it is read, so only rings are needed
                nc.vector.memset(B0[64:128, :, :], 0.0)
                for B in (B0, B1):
                    # top ring = -t_e/s_e (rows 0:64), bottom ring = 0
                    nc.gpsimd.tensor_copy(B[:, 0, :], zbB[:])
                    nc.gpsimd.tensor_copy(B[:, PW - 1, :], zbB[:])
                    nc.gpsimd.tensor_copy(
                        B[:, :, 0:1], zbB[:].rearrange("p (w o) -> p w o", o=1))
                    nc.gpsimd.tensor_copy(
                        B[:, :, PW - 1:PW], zbB[:].rearrange("p (w o) -> p w o", o=1))
                Es = [E0, E1]
                Bs = [B0, B1]
                # dstage rings for all t upfront: -t_d/s_d (spatial pad
                # after dec-BN fold); disjoint from interior writes
                for t in range(T):
                    nc.sync.dma_start(dstage[t, :, 0, :], zbD[:])
                    nc.sync.dma_start(dstage[t, :, PW - 1, :], zbD[:])
                    nc.sync.dma_start(
                        dstage[t, :, :, 0:1],
                        zbD[:].rearrange("p (w o) -> p w o", o=1))
                    nc.sync.dma_start(
                        dstage[t, :, :, PW - 1:PW],
                        zbD[:].rearrange("p (w o) -> p w o", o=1))

                def enc_sb(t, k):
                    Ep, En = Es[t % 2], Es[(t + 1) % 2]
                    Bc = Bs[t % 2]
                    r0, c0 = 8 * k, 1024 * k
                    xs = tp.tile([128, 1024], BF16, tag="xs", bufs=2)
                    nc.sync.dma_start(xs[0:27, :], xim_d[t, :, c0:c0 + 1024])
                    pA = pp.tile([128, 1024], F32, tag="pA", bufs=2)
                    pB = pp.tile([128, 1024], F32, tag="pB", bufs=2)
                    for ch, ps in ((0, pA), (1, pB)):
                        cw = slice(128 * ch, 128 * ch + 128)
                        for hf in range(2):
                            rh = r0 + 4 * hf
                            out = ps[:, 512 * hf:512 * hf + 512]
                            for ky in range(3):  # paired taps (ky,0)+(ky,1)
                                nc.tensor.matmul(
                                    out, encp[:, ky, cw],
                                    Ep[:, rh + ky:rh + ky + 4, 0:128],
                                    start=(ky == 0), stop=False)
                            for ky in range(3):  # single taps (ky,2)
                                nc.tensor.matmul(
                                    out, encs[:, ky, cw],
                                    Ep[:, rh + ky:rh + ky + 4, 2:130],
                                    start=False, stop=False)
                            nc.tensor.matmul(
                                out, encx[:, cw], xs[:, 512 * hf:512 * hf + 512],
                                start=False, stop=True)
                    t_if = tp.tile([128, 1024], F32, tag="t_if", bufs=2, name="t_if")
                    t_u = tp.tile([128, 1024], BF16, tag="t_u", bufs=2, name="t_u")
                    v1 = tp.tile([128, 1024], BF16, tag="t_v", bufs=3, name="v1")
                    v2 = tp.tile([128, 1024], BF16, tag="t_v", bufs=3, name="v2")
                    th = tp.tile([128, 1024], BF16, tag="t_v", bufs=3, name="th")
                    cblk = CC[64:128, c0:c0 + 1024]
                    nc.scalar.activation(t_if[:], pA[:], AF.Relu,
                                         bias=aps[:, 0:1], scale=0.2)
                    nc.scalar.activation(t_u[0:64, :], pB[0:64, :], AF.Tanh,
                                         bias=aps[0:64, 1:2], scale=1.0)
                    nc.scalar.activation(t_u[64:128, :], pB[64:128, :], AF.Relu,
                                         bias=aps[64:128, 1:2], scale=0.2)
                    nc.vector.scalar_tensor_tensor(
                        v1[64:128, :], t_if[0:64, :], 1.0, t_u[0:64, :],
                        ALU.min, ALU.mult)
                    nc.vector.scalar_tensor_tensor(
                        v2[64:128, :], t_if[64:128, :], 1.0, cblk,
                        ALU.min, ALU.mult)
                    nc.vector.tensor_tensor(cblk, v1[64:128, :], v2[64:128, :],
                                            ALU.add)
                    nc.scalar.activation(th[64:128, :], cblk, AF.Tanh)
                    # h -> E_next copy1 (strided image write, fused o*tanh(c))
                    nc.vector.scalar_tensor_tensor(
                        En[0:64, r0 + 1:r0 + 9, 1:1 + W],
                        t_u[64:128, :].rearrange("p (a b) -> p a b", a=8), 1.0,
                        th[64:128, :].rearrange("p (a b) -> p a b", a=8),
                        ALU.min, ALU.mult)
                    # copy2: shifted one column left (for paired taps)
                    nc.gpsimd.tensor_copy(
                        En[64:128, r0 + 1:r0 + 9, 0:W],
                        En[0:64, r0 + 1:r0 + 9, 1:1 + W])
                    # B_cur top: h_enc for the decoder (SBUF->SBUF DMA)
                    nc.sync.dma_start(
                        Bc[0:64, r0 + 1:r0 + 9, 1:1 + W],
                        En[0:64, r0 + 1:r0 + 9, 1:1 + W])

                def dec_sb(t, k):
                    Bc, Bn = Bs[t % 2], Bs[(t + 1) % 2]
                    r0, c0 = 8 * k, 1024 * k
                    pA = pp.tile([128, 1024], F32, tag="pA", bufs=2)
                    pB = pp.tile([128, 1024], F32, tag="pB", bufs=2)
                    for ch, ps in ((0, pA), (1, pB)):
                        cw = slice(128 * ch, 128 * ch + 128)
                        for hf in range(2):
                            rh = r0 + 4 * hf
                            out = ps[:, 512 * hf:512 * hf + 512]
                            for s in range(9):
                                ky, kx = s // 3, s % 3
                                nc.tensor.matmul(
                                    out, decw[:, s, cw],
                                    Bc[:, rh + ky:rh + ky + 4, kx:kx + 128],
                                    start=(s == 0), stop=(s == 8))
                    t_if = tp.tile([128, 1024], F32, tag="t_if", bufs=2, name="t_if")
                    t_u = tp.tile([128, 1024], BF16, tag="t_u", bufs=2, name="t_u")
                    v1 = tp.tile([128, 1024], BF16, tag="t_v", bufs=3, name="v1")
                    v2 = tp.tile([128, 1024], BF16, tag="t_v", bufs=3, name="v2")
                    th = tp.tile([128, 1024], BF16, tag="t_v", bufs=3, name="th")
                    cblk = CC[0:64, c0:c0 + 1024]
                    # dec chunk A is [f|i] (host-permuted); tanh/relu ACT
                    # outputs criss-cross partitions so DVE input bases match
                    nc.scalar.activation(t_if[:], pA[:], AF.Relu,
                                         bias=aps[:, 2:3], scale=0.2)
                    nc.scalar.activation(t_u[64:128, :], pB[0:64, :], AF.Tanh,
                                         bias=aps[0:64, 3:4], scale=1.0)
                    nc.scalar.activation(t_u[0:64, :], pB[64:128, :], AF.Relu,
                                         bias=aps[64:128, 3:4], scale=0.2)
                    nc.vector.scalar_tensor_tensor(
                        v1[0:64, :], t_if[64:128, :], 1.0, t_u[64:128, :],
                        ALU.min, ALU.mult)
                    nc.vector.scalar_tensor_tensor(
                        v2[0:64, :], t_if[0:64, :], 1.0, cblk,
                        ALU.min, ALU.mult)
                    nc.vector.tensor_tensor(cblk, v1[0:64, :], v2[0:64, :],
                                            ALU.add)
                    nc.scalar.activation(th[0:64, :], cblk, AF.Tanh)
                    # h_dec -> B_next bottom (strided image write)
                    nc.vector.scalar_tensor_tensor(
                        Bn[64:128, r0 + 1:r0 + 9, 1:1 + W],
                        t_u[0:64, :].rearrange("p (a b) -> p a b", a=8), 1.0,
                        th[0:64, :].rearrange("p (a b) -> p a b", a=8),
                        ALU.min, ALU.mult)
                    # stage h_dec to DRAM for conv3d
                    nc.gpsimd.dma_start(
                        dstage[t, :, r0 + 1:r0 + 9, 1:1 + W],
                        Bn[64:128, r0 + 1:r0 + 9, 1:1 + W])

                for t in range(T):
                    # decoder lags the encoder by 2 superblocks so the
                    # h_enc SBUF->SBUF DMA into B has ~17us to land
                    for k in range(NSB):
                        enc_sb(t, k)
                        if k >= 2:
                            dec_sb(t, k - 2)
                    dec_sb(t, NSB - 2)
                    dec_sb(t, NSB - 1)

            # ---------------- conv3d + sigmoid ----------------
            with tc.tile_pool(name="cp", bufs=1) as cp, \
                 tc.tile_pool(name="cpp", bufs=1, space=bass.MemorySpace.PSUM) as cpp:
                D = [cp.tile([128, PW, PW], BF16, name=f"D{q}") for q in range(5)]
                for q in range(5):
                    eng = (nc.sync, nc.gpsimd, nc.scalar)[q % 3]
                    eng.dma_start(D[q][0:64, :, :], dstage[2 * q, :, :, :])
                    eng.dma_start(D[q][64:128, :, :], dstage[2 * q + 1, :, :, :])
                for k in range(NSB):
                    r0 = 8 * k
                    py = cpp.tile([30, 1024], F32, tag="py", bufs=2)
                    for hf in range(2):
                        rh = r0 + 4 * hf
                        out = py[:, 512 * hf:512 * hf + 512]
                        i = 0
                        for q in range(5):
                            for s in range(9):
                                ky, kx = s // 3, s % 3
                                nc.tensor.matmul(
                                    out, w3t[:, q * 9 + s, :],
                                    D[q][:, rh + ky:rh + ky + 4, kx:kx + 128],
                                    start=(i == 0), stop=(i == 44))
                                i += 1
                    ty = cp.tile([30, 1024], F32, tag="ty", bufs=2)
                    nc.scalar.activation(ty[:], py[:], AF.Sigmoid,
                                         bias=bout[:], scale=1.0)
                    nc.scalar.dma_start(
                        y_d[:, r0:r0 + 8, :],
                        ty[:].rearrange("p (a b) -> p a b", a=8))

    if do_split:
        split_multi_waits(nc)
    nc.finalize()
    return nc


def _prep(inputs):
    x = np.asarray(inputs["x"], np.float32)
    xpad = np.zeros((8, T, PW, PW, C), np.float32)
    xpad[:, :, 1:1 + H, 1:1 + W, :] = x
    xim = np.empty((8, T, 27, NPIX), BFP)
    for ky in range(3):
        for kx in range(3):
            s = ky * 3 + kx
            v = xpad[:, :, ky:ky + H, kx:kx + W, :]
            xim[:, :, s * 3:s * 3 + 3, :] = (
                v.transpose(0, 1, 4, 2, 3).reshape(8, T, 3, NPIX).astype(BFP))

    enc_Wh = np.asarray(inputs["enc_Wh"], np.float32)
    enc_Wx = np.asarray(inputs["enc_Wx"], np.float32)
    dec_Wx = np.asarray(inputs["dec_Wx"], np.float32)
    dec_Wh = np.asarray(inputs["dec_Wh"], np.float32)
    out_W = np.asarray(inputs["out_W"], np.float32)
    enc_b = np.asarray(inputs["enc_b"], np.float32)
    dec_b = np.asarray(inputs["dec_b"], np.float32)

    # BN affine folds
    s_e = np.asarray(inputs["enc_gamma"], np.float32) / np.sqrt(
        np.asarray(inputs["enc_var"], np.float32) + BN_EPS)
    t_e = np.asarray(inputs["enc_beta"], np.float32) - np.asarray(
        inputs["enc_mean"], np.float32) * s_e
    s_d = np.asarray(inputs["dec_gamma"], np.float32) / np.sqrt(
        np.asarray(inputs["dec_var"], np.float32) + BN_EPS)
    t_d = np.asarray(inputs["dec_beta"], np.float32) - np.asarray(
        inputs["dec_mean"], np.float32) * s_d

    # encoder hidden-conv weights: paired taps (ky,0)+(ky,1), singles (ky,2)
    encp = np.zeros((128, 3, 256), np.float32)
    encs = np.zeros((128, 3, 256), np.float32)
    for ky in range(3):
        encp[0:64, ky, :] = enc_Wh[ky, 0]
        encp[64:128, ky, :] = enc_Wh[ky, 1]
        encs[0:64, ky, :] = enc_Wh[ky, 2]
    encx = np.zeros((128, 256), np.float32)
    encx[0:27, :] = enc_Wx.reshape(27, 256)

    # decoder weights: rows 0:64 dec_Wx scaled by s_e (enc-BN fold),
    # rows 64:128 dec_Wh; gate chunk A permuted to [f|i] so DVE two-input
    # ops get matching base partitions
    perm = np.concatenate([np.arange(64, 128), np.arange(0, 64),
                           np.arange(128, 256)])
    decw = np.zeros((128, 9, 256), np.float32)
    for s in range(9):
        ky, kx = s // 3, s % 3
        decw[0:64, s, :] = (dec_Wx[ky, kx] * s_e[:, None])[:, perm]
        decw[64:128, s, :] = dec_Wh[ky, kx][:, perm]
    # enc-BN shift folded into decoder bias (interior taps; border taps
    # cancelled by the -t_e/s_e ring in B's top half)
    dec_b_f = dec_b + np.einsum("ykcg,c->g", dec_Wx.reshape(3, 3, 64, 256), t_e)

    # conv3d weights scaled by s_d (dec-BN fold)
    out_W_s = out_W * s_d[None, None, None, :, None]
    w3 = np.zeros((45, 128, 30), np.float32)
    for q in range(5):
        for j in range(2):
            f = 2 * q + j
            for t in range(max(0, f - 1), min(T - 1, f + 1) + 1):
                dt = f - t + 1
                for s in range(9):
                    ky, kx = s // 3, s % 3
                    w3[q * 9 + s, 64 * j:64 * j + 64, 3 * t:3 * t + 3] = \
                        out_W_s[dt, ky, kx]
    w3 = w3.transpose(1, 0, 2)  # [128, 45, 30]

    # conv3d bias: out_b + dec-BN shift over valid temporal taps
    out_b = np.asarray(inputs["out_b"], np.float32)
    bout = np.zeros((30,), np.float32)
    for t in range(T):
        acc = out_b.copy()
        for dt in range(3):
            g = t + dt - 1
            if 0 <= g < T:
                acc = acc + np.einsum("ykcg,c->g", out_W[dt], t_d)
        bout[3 * t:3 * t + 3] = acc
    bout = bout.reshape(30, 1)

    aps = np.zeros((128, 4), np.float32)
    aps[0:64, 0] = 0.2 * enc_b[0:64] + 0.5        # enc i
    aps[64:128, 0] = 0.2 * enc_b[64:128] + 0.5    # enc f
    aps[0:64, 1] = enc_b[128:192]                 # enc c~ (tanh bias)
    aps[64:128, 1] = 0.2 * enc_b[192:256] + 0.5   # enc o
    aps[0:64, 2] = 0.2 * dec_b_f[64:128] + 0.5    # dec f (chunk A is [f|i])
    aps[64:128, 2] = 0.2 * dec_b_f[0:64] + 0.5    # dec i
    aps[0:64, 3] = dec_b_f[128:192]               # dec c~
    aps[64:128, 3] = 0.2 * dec_b_f[192:256] + 0.5 # dec o

    ringb = np.zeros((128, PW), np.float32)
    ringb[0:64, :] = (-t_e / s_e)[:, None]
    ringd = np.tile((-t_d / s_d)[:, None], (1, PW))

    shared = {
        "encp": encp.reshape(128, 3 * 256).astype(BFP),
        "encs": encs.reshape(128, 3 * 256).astype(BFP),
        "encx": encx.astype(BFP),
        "decw": decw.reshape(128, 9 * 256).astype(BFP),
        "w3": w3.reshape(128, 45 * 30).astype(BFP),
        "aps": aps,
        "bout": bout,
        "ringb": ringb.astype(BFP),
        "ringd": ringd.astype(BFP),
    }
    return [dict(shared, xim=np.ascontiguousarray(xim[c])) for c in range(8)]


_CACHE = {}


def kernel(**inputs):
    if "nc" not in _CACHE:
        _CACHE["nc"] = _build()
    nc = _CACHE["nc"]
    in_maps = _prep(inputs)
    from concourse.bass_utils import run_bass_kernel_spmd
    res = run_bass_kernel_spmd(nc, in_maps, core_ids=list(range(8)))
    kernel.last_exec_ns = res.exec_time_ns
    kernel.last_res = res
    y = np.stack([
        np.asarray(res.results[c]["y"], np.float32)
        .reshape(T, 3, H, W).transpose(0, 2, 3, 1)
        for c in range(8)
    ])
    return y


# revision 20
# speedup vs baseline: 2.0187x; 1.0126x over previous
"""PredRNN (ConvLSTM enc -> BN -> ConvLSTM dec -> BN -> Conv3D -> sigmoid) on 8 trn2 cores.

Sharding: data-parallel over batch (B=8), one sample per core. Per core:
- Both BNs folded into downstream conv weights; pad-ring constants fix the
  zero-padded borders, so no BN compute on device.
- Encoder hidden conv: h stored twice in the partition dim (plain + shifted
  one column left), so 2 of the 9 taps fuse into one K=128 matmul; all
  encoder matmuls are K=128 (zero-padded weights) to keep the PE HAM clock
  warm (K<128 rhs measured at half clock).
- Decoder: K=128 [h_enc | h_dec] stacked, 9 taps x 2 gate-chunks.
- N=1024 superblocks (8 image rows, 2 PSUM banks per gate-chunk) to amortize
  ACT/DVE fixed overheads; gates evicted with fused hard-sigmoid.
- Conv3D via frame-pair K=128 matmuls with host-baked lhsT, sigmoid eviction.
"""
import sys

sys.path.insert(0, "/opt/trn_rl_repo")
import numpy as np
import ml_dtypes

import concourse.bass as bass
import concourse.tile as tile
from concourse import mybir
from concourse.vector_clock import ScopedClock

BF16 = mybir.dt.bfloat16
F32 = mybir.dt.float32
AF = mybir.ActivationFunctionType
ALU = mybir.AluOpType

T, H, W, F, C = 10, 128, 128, 64, 3
PW = H + 2
NSB = H // 8          # 16 superblocks of 8 rows / 1024 px
NPIX = H * W
BN_EPS = 1e-3
BFP = ml_dtypes.bfloat16


def _patched_drain_and_barrier(self, tick_clock, wait_clock):
    nc = self.nc
    carrier = nc.sync.nop(nofuse=True, hint="drain_waits")
    wait_clock.add_sem_waits(carrier.ins, ScopedClock({None: tick_clock.global_clock}))
    si = carrier.ins.sync_info
    waits = list(si.on_wait) if si is not None else []
    if len(waits) > 1:
        si.on_wait = waits[:1]
        for w in waits[1:]:
            n = nc.sync.nop(nofuse=True, hint="drain_waits")
            n.ins.sync_info = mybir.SyncInfo(on_wait=[w], on_update=[])
    nc.sync.drain()
    nc.all_engine_barrier()
    popped = nc._tile_sem_poison_stack.pop()
    assert popped is self._sem_poison
    nc.clear_and_free_semaphores(list(self.sems.allocated().values()))
    nc.all_engine_barrier()


tile.TileContext._drain_and_barrier = _patched_drain_and_barrier


def split_multi_waits(nc, max_keep=1):
    """Walrus codegen rejects >1 sem wait on compute instructions; hoist
    extras onto same-engine single-wait NOPs inserted just before."""
    n_split = 0
    for fn in nc.m.functions:
        for blk in fn.blocks:
            insts = blk.instructions
            i = 0
            while i < len(insts):
                inst = insts[i]
                si = inst.sync_info
                waits = list(si.on_wait) if si is not None and si.on_wait else []
                if len(waits) > max_keep:
                    for j, w in enumerate(waits[:-max_keep]):
                        nop = mybir.InstNoOp(
                            name=f"{inst.name}_w{j}",
                            engine=inst.engine,
                            sync_info=mybir.SyncInfo(on_wait=[w], on_update=[]),
                            bass_nofuse=True,
                            ins=[],
                            outs=[],
                        )
                        insts.insert(i, nop)
                        i += 1
                    si.on_wait = waits[-max_keep:]
                    n_split += 1
                i += 1
    return n_split


def _build(do_split=True):
    nc = bass.Bass()
    xim_d = nc.dram_tensor("xim", [T, 27, NPIX], BF16, kind="ExternalInput")
    encp_d = nc.dram_tensor("encp", [128, 3 * 256], BF16, kind="ExternalInput")
    encs_d = nc.dram_tensor("encs", [128, 3 * 256], BF16, kind="ExternalInput")
    encx_d = nc.dram_tensor("encx", [128, 256], BF16, kind="ExternalInput")
    decw_d = nc.dram_tensor("decw", [128, 9 * 256], BF16, kind="ExternalInput")
    w3_d = nc.dram_tensor("w3", [128, 15 * 96], BF16, kind="ExternalInput")
    aps_d = nc.dram_tensor("aps", [128, 4], F32, kind="ExternalInput")
    bout_d = nc.dram_tensor("bout", [30, 1], F32, kind="ExternalInput")
    ringb_d = nc.dram_tensor("ringb", [128, PW], BF16, kind="ExternalInput")
    ringd_d = nc.dram_tensor("ringd", [64, PW], BF16, kind="ExternalInput")
    dstage = nc.dram_tensor("dstage", [T, F, PW, PW], BF16, kind="Internal")
    y_d = nc.dram_tensor("y", [30, H, W], F32, kind="ExternalOutput")

    with tile.TileContext(nc) as tc:
        with tc.tile_pool(name="wp", bufs=1) as wp:
            encp = wp.tile([128, 3, 256], BF16)
            encs = wp.tile([128, 3, 256], BF16)
            encx = wp.tile([128, 256], BF16)
            decw = wp.tile([128, 9, 256], BF16)
            w3t = wp.tile([128, 15, 96], BF16)
            aps = wp.tile([128, 4], F32)
            bout = wp.tile([30, 1], F32)
            zbB = wp.tile([128, PW], BF16)
            zbD = wp.tile([64, PW], BF16)
            nc.sync.dma_start(encp[:], encp_d[:].rearrange("p (s g) -> p s g", s=3))
            nc.sync.dma_start(encs[:], encs_d[:].rearrange("p (s g) -> p s g", s=3))
            nc.sync.dma_start(encx[:], encx_d[:])
            nc.sync.dma_start(decw[:], decw_d[:].rearrange("p (s g) -> p s g", s=9))
            nc.sync.dma_start(w3t[:], w3_d[:].rearrange("p (s g) -> p s g", s=15))
            nc.sync.dma_start(aps[:], aps_d[:])
            nc.sync.dma_start(bout[:], bout_d[:])
            nc.sync.dma_start(zbB[:], ringb_d[:])
            nc.sync.dma_start(zbD[:], ringd_d[:])

            with tc.tile_pool(name="sp", bufs=1) as sp, \
                 tc.tile_pool(name="tp", bufs=1) as tp, \
                 tc.tile_pool(name="pp", bufs=1, space=bass.MemorySpace.PSUM) as pp:
                E0 = sp.tile([128, PW, PW], BF16)
                E1 = sp.tile([128, PW, PW], BF16)
                B0 = sp.tile([128, PW, PW], BF16)
                B1 = sp.tile([128, PW, PW], BF16)
                CC = sp.tile([128, NPIX], BF16)
                # init xs staging buffers to zero first (rows 27:128 stay
                # zero, multiplied by zero weight rows -> avoids NaN garbage)
                for _ in range(2):
                    xs0 = tp.tile([128, 1024], BF16, tag="xs", bufs=2, name="xs0")
                    nc.vector.memset(xs0[:], 0.0)
                nc.vector.memset(E0[:], 0.0)   # h_enc(-1) = 0
                nc.vector.memset(CC[:], 0.0)   # c(-1) = 0
                # E1 only needs its pad ring zeroed (interior fully written
                # at t=0); copy2 tail cols 128:130 also never written
                nc.vector.memset(E1[:, 0, :], 0.0)
                nc.vector.memset(E1[:, PW - 1, :], 0.0)
                nc.vector.memset(E1[:, :, 0:1], 0.0)
                nc.vector.memset(E1[:, :, PW - 2:PW], 0.0)
                # B0 bottom half: h_dec(-1) = 0; B1 interior is written
                # before it is read, so only rings are needed
                nc.vector.memset(B0[64:128, :, :], 0.0)
                for B in (B0, B1):
                    # top ring = -t_e/s_e (rows 0:64), bottom ring = 0
                    nc.gpsimd.tensor_copy(B[:, 0, :], zbB[:])
                    nc.gpsimd.tensor_copy(B[:, PW - 1, :], zbB[:])
                    nc.gpsimd.tensor_copy(
                        B[:, :, 0:1], zbB[:].rearrange("p (w o) -> p w o", o=1))
                    nc.gpsimd.tensor_copy(
                        B[:, :, PW - 1:PW], zbB[:].rearrange("p (w o) -> p w o", o=1))
                Es = [E0, E1]
                Bs = [B0, B1]

                def enc_sb(t, k):
                    Ep, En = Es[t % 2], Es[(t + 1) % 2]
                    Bc = Bs[t % 2]
                    r0, c0 = 8 * k, 1024 * k
                    xs = tp.tile([128, 1024], BF16, tag="xs", bufs=2)
                    nc.sync.dma_start(xs[0:27, :], xim_d[t, :, c0:c0 + 1024])
                    pA = pp.tile([128, 1024], F32, tag="pA", bufs=2)
                    pB = pp.tile([128, 1024], F32, tag="pB", bufs=2)
                    for ch, ps in ((0, pA), (1, pB)):
                        cw = slice(128 * ch, 128 * ch + 128)
                        for hf in range(2):
                            rh = r0 + 4 * hf
                            out = ps[:, 512 * hf:512 * hf + 512]
                            for ky in range(3):  # paired taps (ky,0)+(ky,1)
                                nc.tensor.matmul(
                                    out, encp[:, ky, cw],
                                    Ep[:, rh + ky:rh + ky + 4, 0:128],
                                    start=(ky == 0), stop=False)
                            for ky in range(3):  # single taps (ky,2)
                                nc.tensor.matmul(
                                    out, encs[:, ky, cw],
                                    Ep[:, rh + ky:rh + ky + 4, 2:130],
                                    start=False, stop=False)
                            nc.tensor.matmul(
                                out, encx[:, cw], xs[:, 512 * hf:512 * hf + 512],
                                start=False, stop=True)
                    t_if = tp.tile([128, 1024], F32, tag="t_if", bufs=2, name="t_if")
                    t_u = tp.tile([128, 1024], BF16, tag="t_u", bufs=2, name="t_u")
                    v1 = tp.tile([128, 1024], BF16, tag="t_v", bufs=3, name="v1")
                    v2 = tp.tile([128, 1024], BF16, tag="t_v", bufs=3, name="v2")
                    th = tp.tile([128, 1024], BF16, tag="t_v", bufs=3, name="th")
                    cblk = CC[64:128, c0:c0 + 1024]
                    nc.scalar.activation(t_if[:], pA[:], AF.Relu,
                                         bias=aps[:, 0:1], scale=0.2)
                    nc.scalar.activation(t_u[0:64, :], pB[0:64, :], AF.Tanh,
                                         bias=aps[0:64, 1:2], scale=1.0)
                    nc.scalar.activation(t_u[64:128, :], pB[64:128, :], AF.Relu,
                                         bias=aps[64:128, 1:2], scale=0.2)
                    nc.vector.scalar_tensor_tensor(
                        v1[64:128, :], t_if[0:64, :], 1.0, t_u[0:64, :],
                        ALU.min, ALU.mult)
                    nc.vector.scalar_tensor_tensor(
                        v2[64:128, :], t_if[64:128, :], 1.0, cblk,
                        ALU.min, ALU.mult)
                    nc.vector.tensor_tensor(cblk, v1[64:128, :], v2[64:128, :],
                                            ALU.add)
                    nc.scalar.activation(th[64:128, :], cblk, AF.Tanh)
                    # h -> E_next copy1 (strided image write, fused o*tanh(c))
                    nc.vector.scalar_tensor_tensor(
                        En[0:64, r0 + 1:r0 + 9, 1:1 + W],
                        t_u[64:128, :].rearrange("p (a b) -> p a b", a=8), 1.0,
                        th[64:128, :].rearrange("p (a b) -> p a b", a=8),
                        ALU.min, ALU.mult)
                    # copy2: shifted one column left (for paired taps)
                    nc.gpsimd.tensor_copy(
                        En[64:128, r0 + 1:r0 + 9, 0:W],
                        En[0:64, r0 + 1:r0 + 9, 1:1 + W])
                    # B_cur top: h_enc for the decoder (SBUF->SBUF DMA)
                    nc.sync.dma_start(
                        Bc[0:64, r0 + 1:r0 + 9, 1:1 + W],
                        En[0:64, r0 + 1:r0 + 9, 1:1 + W])

                def dec_sb(t, k):
                    Bc, Bn = Bs[t % 2], Bs[(t + 1) % 2]
                    r0, c0 = 8 * k, 1024 * k
                    pA = pp.tile([128, 1024], F32, tag="pA", bufs=2)
                    pB = pp.tile([128, 1024], F32, tag="pB", bufs=2)
                    for ch, ps in ((0, pA), (1, pB)):
                        cw = slice(128 * ch, 128 * ch + 128)
                        for hf in range(2):
                            rh = r0 + 4 * hf
                            out = ps[:, 512 * hf:512 * hf + 512]
                            for s in range(9):
                                ky, kx = s // 3, s % 3
                                nc.tensor.matmul(
                                    out, decw[:, s, cw],
                                    Bc[:, rh + ky:rh + ky + 4, kx:kx + 128],
                                    start=(s == 0), stop=(s == 8))
                    t_if = tp.tile([128, 1024], F32, tag="t_if", bufs=2, name="t_if")
                    t_u = tp.tile([128, 1024], BF16, tag="t_u", bufs=2, name="t_u")
                    v1 = tp.tile([128, 1024], BF16, tag="t_v", bufs=3, name="v1")
                    v2 = tp.tile([128, 1024], BF16, tag="t_v", bufs=3, name="v2")
                    th = tp.tile([128, 1024], BF16, tag="t_v", bufs=3, name="th")
                    cblk = CC[0:64, c0:c0 + 1024]
                    # dec chunk A is [f|i] (host-permuted); tanh/relu ACT
                    # outputs criss-cross partitions so DVE input bases match
                    nc.scalar.activation(t_if[:], pA[:], AF.Relu,
                                         bias=aps[:, 2:3], scale=0.2)
                    nc.scalar.activation(t_u[64:128, :], pB[0:64, :], AF.Tanh,
                                         bias=aps[0:64, 3:4], scale=1.0)
                    nc.scalar.activation(t_u[0:64, :], pB[64:128, :], AF.Relu,
                                         bias=aps[64:128, 3:4], scale=0.2)
                    nc.vector.scalar_tensor_tensor(
                        v1[0:64, :], t_if[64:128, :], 1.0, t_u[64:128, :],
                        ALU.min, ALU.mult)
                    nc.vector.scalar_tensor_tensor(
                        v2[0:64, :], t_if[0:64, :], 1.0, cblk,
                        ALU.min, ALU.mult)
                    nc.vector.tensor_tensor(cblk, v1[0:64, :], v2[0:64, :],
                                            ALU.add)
                    nc.scalar.activation(th[0:64, :], cblk, AF.Tanh)
                    # h_dec -> B_next bottom (strided image write)
                    nc.vector.scalar_tensor_tensor(
                        Bn[64:128, r0 + 1:r0 + 9, 1:1 + W],
                        t_u[0:64, :].rearrange("p (a b) -> p a b", a=8), 1.0,
                        th[0:64, :].rearrange("p (a b) -> p a b", a=8),
                        ALU.min, ALU.mult)
                    # stage h_dec to DRAM for conv3d
                    nc.gpsimd.dma_start(
                        dstage[t, :, r0 + 1:r0 + 9, 1:1 + W],
                        Bn[64:128, r0 + 1:r0 + 9, 1:1 + W])

                for t in range(T):
                    # dstage[t] pad ring = -t_d/s_d (dec-BN fold); scalar
                    # queue is idle for DMA and cannot block xs loads
                    nc.scalar.dma_start(dstage[t, :, 0, :], zbD[:])
                    nc.scalar.dma_start(dstage[t, :, PW - 1, :], zbD[:])
                    nc.scalar.dma_start(
                        dstage[t, :, :, 0:1],
                        zbD[:].rearrange("p (w o) -> p w o", o=1))
                    nc.scalar.dma_start(
                        dstage[t, :, :, PW - 1:PW],
                        zbD[:].rearrange("p (w o) -> p w o", o=1))
                    # decoder lags the encoder by 2 superblocks so the
                    # h_enc SBUF->SBUF DMA into B has ~17us to land
                    for k in range(NSB):
                        enc_sb(t, k)
                        if k >= 2:
                            dec_sb(t, k - 2)
                    dec_sb(t, NSB - 2)
                    dec_sb(t, NSB - 1)

            # ---------------- conv3d + sigmoid ----------------
            # ky folded into M (90 = 3ky x 30 outputs): 15 matmuls per
            # 4-row window; ky-shifted partial sums combined on DVE.
            with tc.tile_pool(name="cp", bufs=1) as cp, \
                 tc.tile_pool(name="cpp", bufs=1, space=bass.MemorySpace.PSUM) as cpp:
                D = [cp.tile([128, PW, PW], BF16, name=f"D{q}") for q in range(5)]
                for q in range(5):
                    for j in range(2):
                        eng = (nc.sync, nc.gpsimd, nc.scalar)[(2 * q + j) % 3]
                        eng.dma_start(D[q][64 * j:64 * j + 64, :, :],
                                      dstage[2 * q + j, :, :, :])

                def evict(w):
                    # window w streams D_pad rows w..w+nw-1
                    nw = 2 if w == H else 4
                    py = cpp.tile([96, 512], F32, tag="py", bufs=3, name="py")
                    for q in range(5):
                        for kx in range(3):
                            i = 3 * q + kx
                            nc.tensor.matmul(
                                py[:, 0:128 * nw], w3t[:, i, :],
                                D[q][:, w:w + nw, kx:kx + 128],
                                start=(i == 0), stop=(i == 14))
                    shA = cp.tile([32, 512], F32, tag="shA", bufs=2, name="shA")
                    shB = cp.tile([32, 512], F32, tag="shB", bufs=2, name="shB")
                    shC = cp.tile([32, 512], F32, tag="shC", bufs=2, name="shC")
                    nc.scalar.activation(shA[:, 0:128 * nw], py[0:32, 0:128 * nw],
                                         AF.Copy)
                    nc.scalar.activation(shB[:, 0:128 * nw], py[32:64, 0:128 * nw],
                                         AF.Copy)
                    nc.scalar.activation(shC[:, 0:128 * nw], py[64:96, 0:128 * nw],
                                         AF.Copy)
                    return shA, shB, shC

                prev = evict(0)
                for k in range(NSB * 2):
                    y0 = 4 * k
                    cur, prev = prev, evict(y0 + 4)
                    cA, cB, cC = cur
                    nA, nB, nC = prev
                    u = cp.tile([30, 4, 128], F32, tag="u", bufs=2, name="u")
                    v = cp.tile([30, 4, 128], F32, tag="v", bufs=2, name="v")
                    cA = cA[0:30, :].rearrange("p (a b) -> p a b", b=128)
                    cB = cB[0:30, :].rearrange("p (a b) -> p a b", b=128)
                    cC = cC[0:30, :].rearrange("p (a b) -> p a b", b=128)
                    nB = nB[0:30, :].rearrange("p (a b) -> p a b", b=128)
                    nC = nC[0:30, :].rearrange("p (a b) -> p a b", b=128)
                    nc.vector.tensor_tensor(u[:, 0:3, :], cA[:, 0:3, :],
                                            cB[:, 1:4, :], ALU.add)
                    nc.vector.tensor_tensor(u[:, 3:4, :], cA[:, 3:4, :],
                                            nB[:, 0:1, :], ALU.add)
                    nc.vector.tensor_tensor(v[:, 0:2, :], u[:, 0:2, :],
                                            cC[:, 2:4, :], ALU.add)
                    nc.vector.tensor_tensor(v[:, 2:4, :], u[:, 2:4, :],
                                            nC[:, 0:2, :], ALU.add)
                    ty = cp.tile([30, 512], F32, tag="ty", bufs=2, name="ty")
                    nc.scalar.activation(ty[:], v[:].rearrange("p a b -> p (a b)"),
                                         AF.Sigmoid, bias=bout[:], scale=1.0)
                    nc.scalar.dma_start(
                        y_d[:, y0:y0 + 4, :],
                        ty[:].rearrange("p (a b) -> p a b", a=4))

    if do_split:
        split_multi_waits(nc)
    nc.finalize()
    return nc


def _prep(inputs):
    x = np.asarray(inputs["x"], np.float32)
    xpad = np.zeros((8, T, PW, PW, C), np.float32)
    xpad[:, :, 1:1 + H, 1:1 + W, :] = x
    xim = np.empty((8, T, 27, NPIX), BFP)
    for ky in range(3):
        for kx in range(3):
            s = ky * 3 + kx
            v = xpad[:, :, ky:ky + H, kx:kx + W, :]
            xim[:, :, s * 3:s * 3 + 3, :] = (
                v.transpose(0, 1, 4, 2, 3).reshape(8, T, 3, NPIX).astype(BFP))

    enc_Wh = np.asarray(inputs["enc_Wh"], np.float32)
    enc_Wx = np.asarray(inputs["enc_Wx"], np.float32)
    dec_Wx = np.asarray(inputs["dec_Wx"], np.float32)
    dec_Wh = np.asarray(inputs["dec_Wh"], np.float32)
    out_W = np.asarray(inputs["out_W"], np.float32)
    enc_b = np.asarray(inputs["enc_b"], np.float32)
    dec_b = np.asarray(inputs["dec_b"], np.float32)

    # BN affine folds
    s_e = np.asarray(inputs["enc_gamma"], np.float32) / np.sqrt(
        np.asarray(inputs["enc_var"], np.float32) + BN_EPS)
    t_e = np.asarray(inputs["enc_beta"], np.float32) - np.asarray(
        inputs["enc_mean"], np.float32) * s_e
    s_d = np.asarray(inputs["dec_gamma"], np.float32) / np.sqrt(
        np.asarray(inputs["dec_var"], np.float32) + BN_EPS)
    t_d = np.asarray(inputs["dec_beta"], np.float32) - np.asarray(
        inputs["dec_mean"], np.float32) * s_d

    # encoder hidden-conv weights: paired taps (ky,0)+(ky,1), singles (ky,2)
    encp = np.zeros((128, 3, 256), np.float32)
    encs = np.zeros((128, 3, 256), np.float32)
    for ky in range(3):
        encp[0:64, ky, :] = enc_Wh[ky, 0]
        encp[64:128, ky, :] = enc_Wh[ky, 1]
        encs[0:64, ky, :] = enc_Wh[ky, 2]
    encx = np.zeros((128, 256), np.float32)
    encx[0:27, :] = enc_Wx.reshape(27, 256)

    # decoder weights: rows 0:64 dec_Wx scaled by s_e (enc-BN fold),
    # rows 64:128 dec_Wh; gate chunk A permuted to [f|i] so DVE two-input
    # ops get matching base partitions
    perm = np.concatenate([np.arange(64, 128), np.arange(0, 64),
                           np.arange(128, 256)])
    decw = np.zeros((128, 9, 256), np.float32)
    for s in range(9):
        ky, kx = s // 3, s % 3
        decw[0:64, s, :] = (dec_Wx[ky, kx] * s_e[:, None])[:, perm]
        decw[64:128, s, :] = dec_Wh[ky, kx][:, perm]
    # enc-BN shift folded into decoder bias (interior taps; border taps
    # cancelled by the -t_e/s_e ring in B's top half)
    dec_b_f = dec_b + np.einsum("ykcg,c->g", dec_Wx.reshape(3, 3, 64, 256), t_e)

    # conv3d weights scaled by s_d (dec-BN fold); ky folded into M:
    # col 32*ky + (3t+c), 32-row groups for partition-base alignment
    out_W_s = out_W * s_d[None, None, None, :, None]
    w3 = np.zeros((15, 128, 96), np.float32)
    for q in range(5):
        for j in range(2):
            f = 2 * q + j
            for t in range(max(0, f - 1), min(T - 1, f + 1) + 1):
                dt = f - t + 1
                for ky in range(3):
                    for kx in range(3):
                        w3[q * 3 + kx, 64 * j:64 * j + 64,
                           32 * ky + 3 * t:32 * ky + 3 * t + 3] = \
                            out_W_s[dt, ky, kx]
    w3 = w3.transpose(1, 0, 2)  # [128, 15, 96]

    # conv3d bias: out_b + dec-BN shift over valid temporal taps
    out_b = np.asarray(inputs["out_b"], np.float32)
    bout = np.zeros((30,), np.float32)
    for t in range(T):
        acc = out_b.copy()
        for dt in range(3):
            g = t + dt - 1
            if 0 <= g < T:
                acc = acc + np.einsum("ykcg,c->g", out_W[dt], t_d)
        bout[3 * t:3 * t + 3] = acc
    bout = bout.reshape(30, 1)

    aps = np.zeros((128, 4), np.float32)
    aps[0:64, 0] = 0.2 * enc_b[0:64] + 0.5        # enc i
    aps[64:128, 0] = 0.2 * enc_b[64:128] + 0.5    # enc f
    aps[0:64, 1] = enc_b[128:192]                 # enc c~ (tanh bias)
    aps[64:128, 1] = 0.2 * enc_b[192:256] + 0.5   # enc o
    aps[0:64, 2] = 0.2 * dec_b_f[64:128] + 0.5    # dec f (chunk A is [f|i])
    aps[64:128, 2] = 0.2 * dec_b_f[0:64] + 0.5    # dec i
    aps[0:64, 3] = dec_b_f[128:192]               # dec c~
    aps[64:128, 3] = 0.2 * dec_b_f[192:256] + 0.5 # dec o

    ringb = np.zeros((128, PW), np.float32)
    ringb[0:64, :] = (-t_e / s_e)[:, None]
    ringd = np.tile((-t_d / s_d)[:, None], (1, PW))

    shared = {
        "encp": encp.reshape(128, 3 * 256).astype(BFP),
        "encs": encs.reshape(128, 3 * 256).astype(BFP),
        "encx": encx.astype(BFP),
        "decw": decw.reshape(128, 9 * 256).astype(BFP),
        "w3": w3.reshape(128, 15 * 96).astype(BFP),
        "aps": aps,
        "bout": bout,
        "ringb": ringb.astype(BFP),
        "ringd": ringd.astype(BFP),
    }
    return [dict(shared, xim=np.ascontiguousarray(xim[c])) for c in range(8)]


_CACHE = {}


def kernel(**inputs):
    if "nc" not in _CACHE:
        _CACHE["nc"] = _build()
    nc = _CACHE["nc"]
    in_maps = _prep(inputs)
    from concourse.bass_utils import run_bass_kernel_spmd
    res = run_bass_kernel_spmd(nc, in_maps, core_ids=list(range(8)))
    kernel.last_exec_ns = res.exec_time_ns
    kernel.last_res = res
    y = np.stack([
        np.asarray(res.results[c]["y"], np.float32)
        .reshape(T, 3, H, W).transpose(0, 2, 3, 1)
        for c in range(8)
    ])
    return y


# revision 22
# speedup vs baseline: 2.3067x; 1.1427x over previous
"""PredRNN (ConvLSTM enc -> BN -> ConvLSTM dec -> BN -> Conv3D -> sigmoid) on 8 trn2 cores.

Sharding: data-parallel over batch (B=8), one sample per core. Per core:
- Both BNs folded into downstream conv weights; pad-ring constants fix the
  zero-padded borders, so no BN compute on device.
- Encoder hidden conv: h stored twice in the partition dim (plain + shifted
  one column left), so 2 of the 9 taps fuse into one K=128 matmul; all
  encoder matmuls are K=128 (zero-padded weights) to keep the PE HAM clock
  warm (K<128 rhs measured at half clock).
- Decoder: K=128 [h_enc | h_dec] stacked, 9 taps x 2 gate-chunks.
- N=1024 superblocks (8 image rows, 2 PSUM banks per gate-chunk) to amortize
  ACT/DVE fixed overheads; gates evicted with fused hard-sigmoid.
- Conv3D via frame-pair K=128 matmuls with host-baked lhsT, sigmoid eviction.
"""
import sys

sys.path.insert(0, "/opt/trn_rl_repo")
import numpy as np
import ml_dtypes

import concourse.bass as bass
import concourse.tile as tile
from concourse import mybir
from concourse.vector_clock import ScopedClock

BF16 = mybir.dt.bfloat16
F32 = mybir.dt.float32
AF = mybir.ActivationFunctionType
ALU = mybir.AluOpType

T, H, W, F, C = 10, 128, 128, 64, 3
PW = H + 2
NSB = H // 8          # 16 superblocks of 8 rows / 1024 px
NPIX = H * W
BN_EPS = 1e-3
BFP = ml_dtypes.bfloat16


def _patched_drain_and_barrier(self, tick_clock, wait_clock):
    nc = self.nc
    carrier = nc.sync.nop(nofuse=True, hint="drain_waits")
    wait_clock.add_sem_waits(carrier.ins, ScopedClock({None: tick_clock.global_clock}))
    si = carrier.ins.sync_info
    waits = list(si.on_wait) if si is not None else []
    if len(waits) > 1:
        si.on_wait = waits[:1]
        for w in waits[1:]:
            n = nc.sync.nop(nofuse=True, hint="drain_waits")
            n.ins.sync_info = mybir.SyncInfo(on_wait=[w], on_update=[])
    nc.sync.drain()
    nc.all_engine_barrier()
    popped = nc._tile_sem_poison_stack.pop()
    assert popped is self._sem_poison
    nc.clear_and_free_semaphores(list(self.sems.allocated().values()))
    nc.all_engine_barrier()


tile.TileContext._drain_and_barrier = _patched_drain_and_barrier


def split_multi_waits(nc, max_keep=1):
    """Walrus codegen rejects >1 sem wait on compute instructions; hoist
    extras onto same-engine single-wait NOPs inserted just before."""
    n_split = 0
    for fn in nc.m.functions:
        for blk in fn.blocks:
            insts = blk.instructions
            i = 0
            while i < len(insts):
                inst = insts[i]
                si = inst.sync_info
                waits = list(si.on_wait) if si is not None and si.on_wait else []
                if len(waits) > max_keep:
                    for j, w in enumerate(waits[:-max_keep]):
                        nop = mybir.InstNoOp(
                            name=f"{inst.name}_w{j}",
                            engine=inst.engine,
                            sync_info=mybir.SyncInfo(on_wait=[w], on_update=[]),
                            bass_nofuse=True,
                            ins=[],
                            outs=[],
                        )
                        insts.insert(i, nop)
                        i += 1
                    si.on_wait = waits[-max_keep:]
                    n_split += 1
                i += 1
    return n_split


def _build(do_split=True):
    nc = bass.Bass()
    xim_d = nc.dram_tensor("xim", [T, 27, NPIX], BF16, kind="ExternalInput")
    encp_d = nc.dram_tensor("encp", [128, 3 * 256], BF16, kind="ExternalInput")
    encs_d = nc.dram_tensor("encs", [128, 3 * 256], BF16, kind="ExternalInput")
    encx_d = nc.dram_tensor("encx", [128, 256], BF16, kind="ExternalInput")
    decw_d = nc.dram_tensor("decw", [128, 9 * 256], BF16, kind="ExternalInput")
    w3_d = nc.dram_tensor("w3", [128, 15 * 96], BF16, kind="ExternalInput")
    aps_d = nc.dram_tensor("aps", [128, 4], F32, kind="ExternalInput")
    bout_d = nc.dram_tensor("bout", [30, 1], F32, kind="ExternalInput")
    ringb_d = nc.dram_tensor("ringb", [128, PW], BF16, kind="ExternalInput")
    ringd_d = nc.dram_tensor("ringd", [64, PW], BF16, kind="ExternalInput")
    colfix_d = nc.dram_tensor("colfix", [30, 6], F32, kind="ExternalInput")
    dstage = nc.dram_tensor("dstage", [T, F, PW, PW], BF16, kind="Internal")
    y_d = nc.dram_tensor("y", [30, H, W], F32, kind="ExternalOutput")

    with tile.TileContext(nc) as tc:
        with tc.tile_pool(name="wp", bufs=1) as wp:
            encp = wp.tile([128, 3, 256], BF16)
            encs = wp.tile([128, 3, 256], BF16)
            encx = wp.tile([128, 256], BF16)
            decw = wp.tile([128, 9, 256], BF16)
            w3t = wp.tile([128, 15, 96], BF16)
            aps = wp.tile([128, 4], F32)
            bout = wp.tile([30, 1], F32)
            zbB = wp.tile([128, PW], BF16)
            zbD = wp.tile([64, PW], BF16)
            colfix = wp.tile([30, 6], F32)
            nc.sync.dma_start(encp[:], encp_d[:].rearrange("p (s g) -> p s g", s=3))
            nc.sync.dma_start(encs[:], encs_d[:].rearrange("p (s g) -> p s g", s=3))
            nc.sync.dma_start(encx[:], encx_d[:])
            nc.sync.dma_start(decw[:], decw_d[:].rearrange("p (s g) -> p s g", s=9))
            nc.sync.dma_start(w3t[:], w3_d[:].rearrange("p (s g) -> p s g", s=15))
            nc.sync.dma_start(aps[:], aps_d[:])
            nc.sync.dma_start(bout[:], bout_d[:])
            nc.sync.dma_start(zbB[:], ringb_d[:])
            nc.sync.dma_start(zbD[:], ringd_d[:])
            nc.sync.dma_start(colfix[:], colfix_d[:])

            with tc.tile_pool(name="sp", bufs=1) as sp, \
                 tc.tile_pool(name="tp", bufs=1) as tp, \
                 tc.tile_pool(name="pp", bufs=1, space=bass.MemorySpace.PSUM) as pp:
                E0 = sp.tile([128, PW, PW], BF16)
                E1 = sp.tile([128, PW, PW], BF16)
                B0 = sp.tile([128, PW, PW], BF16)
                B1 = sp.tile([128, PW, PW], BF16)
                CC = sp.tile([128, NPIX], BF16)
                # init xs staging buffers to zero first (rows 27:128 stay
                # zero, multiplied by zero weight rows -> avoids NaN garbage)
                for _ in range(2):
                    xs0 = tp.tile([128, 1024], BF16, tag="xs", bufs=2, name="xs0")
                    nc.vector.memset(xs0[:], 0.0)
                nc.vector.memset(E0[:], 0.0)   # h_enc(-1) = 0
                nc.vector.memset(CC[:], 0.0)   # c(-1) = 0
                # E1 only needs its pad ring zeroed (interior fully written
                # at t=0); copy2 tail cols 128:130 also never written
                nc.vector.memset(E1[:, 0, :], 0.0)
                nc.vector.memset(E1[:, PW - 1, :], 0.0)
                nc.vector.memset(E1[:, :, 0:1], 0.0)
                nc.vector.memset(E1[:, :, PW - 2:PW], 0.0)
                # B0 bottom half: h_dec(-1) = 0; B1 interior is written
                # before it is read, so only rings are needed
                nc.vector.memset(B0[64:128, :, :], 0.0)
                for B in (B0, B1):
                    # top ring = -t_e/s_e (rows 0:64), bottom ring = 0
                    nc.gpsimd.tensor_copy(B[:, 0, :], zbB[:])
                    nc.gpsimd.tensor_copy(B[:, PW - 1, :], zbB[:])
                    nc.gpsimd.tensor_copy(
                        B[:, :, 0:1], zbB[:].rearrange("p (w o) -> p w o", o=1))
                    nc.gpsimd.tensor_copy(
                        B[:, :, PW - 1:PW], zbB[:].rearrange("p (w o) -> p w o", o=1))
                Es = [E0, E1]
                Bs = [B0, B1]

                def enc_sb(t, k):
                    Ep, En = Es[t % 2], Es[(t + 1) % 2]
                    Bc = Bs[t % 2]
                    r0, c0 = 8 * k, 1024 * k
                    xs = tp.tile([128, 1024], BF16, tag="xs", bufs=2)
                    nc.sync.dma_start(xs[0:27, :], xim_d[t, :, c0:c0 + 1024])
                    pA = pp.tile([128, 1024], F32, tag="pA", bufs=2)
                    pB = pp.tile([128, 1024], F32, tag="pB", bufs=2)
                    for ch, ps in ((0, pA), (1, pB)):
                        cw = slice(128 * ch, 128 * ch + 128)
                        for hf in range(2):
                            rh = r0 + 4 * hf
                            out = ps[:, 512 * hf:512 * hf + 512]
                            for ky in range(3):  # paired taps (ky,0)+(ky,1)
                                nc.tensor.matmul(
                                    out, encp[:, ky, cw],
                                    Ep[:, rh + ky:rh + ky + 4, 0:128],
                                    start=(ky == 0), stop=False)
                            for ky in range(3):  # single taps (ky,2)
                                nc.tensor.matmul(
                                    out, encs[:, ky, cw],
                                    Ep[:, rh + ky:rh + ky + 4, 2:130],
                                    start=False, stop=False)
                            nc.tensor.matmul(
                                out, encx[:, cw], xs[:, 512 * hf:512 * hf + 512],
                                start=False, stop=True)
                    t_if = tp.tile([128, 1024], F32, tag="t_if", bufs=2, name="t_if")
                    t_u = tp.tile([128, 1024], BF16, tag="t_u", bufs=2, name="t_u")
                    v1 = tp.tile([128, 1024], BF16, tag="t_v", bufs=3, name="v1")
                    v2 = tp.tile([128, 1024], BF16, tag="t_v", bufs=3, name="v2")
                    th = tp.tile([128, 1024], BF16, tag="t_v", bufs=3, name="th")
                    cblk = CC[64:128, c0:c0 + 1024]
                    nc.scalar.activation(t_if[:], pA[:], AF.Relu,
                                         bias=aps[:, 0:1], scale=0.2)
                    nc.scalar.activation(t_u[0:64, :], pB[0:64, :], AF.Tanh,
                                         bias=aps[0:64, 1:2], scale=1.0)
                    nc.scalar.activation(t_u[64:128, :], pB[64:128, :], AF.Relu,
                                         bias=aps[64:128, 1:2], scale=0.2)
                    nc.vector.scalar_tensor_tensor(
                        v1[64:128, :], t_if[0:64, :], 1.0, t_u[0:64, :],
                        ALU.min, ALU.mult)
                    nc.vector.scalar_tensor_tensor(
                        v2[64:128, :], t_if[64:128, :], 1.0, cblk,
                        ALU.min, ALU.mult)
                    nc.vector.tensor_tensor(cblk, v1[64:128, :], v2[64:128, :],
                                            ALU.add)
                    nc.scalar.activation(th[64:128, :], cblk, AF.Tanh)
                    # h -> E_next copy1 (strided image write, fused o*tanh(c))
                    nc.vector.scalar_tensor_tensor(
                        En[0:64, r0 + 1:r0 + 9, 1:1 + W],
                        t_u[64:128, :].rearrange("p (a b) -> p a b", a=8), 1.0,
                        th[64:128, :].rearrange("p (a b) -> p a b", a=8),
                        ALU.min, ALU.mult)
                    # copy2: shifted one column left (for paired taps)
                    nc.gpsimd.tensor_copy(
                        En[64:128, r0 + 1:r0 + 9, 0:W],
                        En[0:64, r0 + 1:r0 + 9, 1:1 + W])
                    # B_cur top: h_enc for the decoder (SBUF->SBUF DMA)
                    nc.sync.dma_start(
                        Bc[0:64, r0 + 1:r0 + 9, 1:1 + W],
                        En[0:64, r0 + 1:r0 + 9, 1:1 + W])

                def dec_sb(t, k):
                    Bc, Bn = Bs[t % 2], Bs[(t + 1) % 2]
                    r0, c0 = 8 * k, 1024 * k
                    pA = pp.tile([128, 1024], F32, tag="pA", bufs=2)
                    pB = pp.tile([128, 1024], F32, tag="pB", bufs=2)
                    for ch, ps in ((0, pA), (1, pB)):
                        cw = slice(128 * ch, 128 * ch + 128)
                        for hf in range(2):
                            rh = r0 + 4 * hf
                            out = ps[:, 512 * hf:512 * hf + 512]
                            for s in range(9):
                                ky, kx = s // 3, s % 3
                                nc.tensor.matmul(
                                    out, decw[:, s, cw],
                                    Bc[:, rh + ky:rh + ky + 4, kx:kx + 128],
                                    start=(s == 0), stop=(s == 8))
                    t_if = tp.tile([128, 1024], F32, tag="t_if", bufs=2, name="t_if")
                    t_u = tp.tile([128, 1024], BF16, tag="t_u", bufs=2, name="t_u")
                    v1 = tp.tile([128, 1024], BF16, tag="t_v", bufs=3, name="v1")
                    v2 = tp.tile([128, 1024], BF16, tag="t_v", bufs=3, name="v2")
                    th = tp.tile([128, 1024], BF16, tag="t_v", bufs=3, name="th")
                    cblk = CC[0:64, c0:c0 + 1024]
                    # dec chunk A is [f|i] (host-permuted); tanh/relu ACT
                    # outputs criss-cross partitions so DVE input bases match
                    nc.scalar.activation(t_if[:], pA[:], AF.Relu,
                                         bias=aps[:, 2:3], scale=0.2)
                    nc.scalar.activation(t_u[64:128, :], pB[0:64, :], AF.Tanh,
                                         bias=aps[0:64, 3:4], scale=1.0)
                    nc.scalar.activation(t_u[0:64, :], pB[64:128, :], AF.Relu,
                                         bias=aps[64:128, 3:4], scale=0.2)
                    nc.vector.scalar_tensor_tensor(
                        v1[0:64, :], t_if[64:128, :], 1.0, t_u[64:128, :],
                        ALU.min, ALU.mult)
                    nc.vector.scalar_tensor_tensor(
                        v2[0:64, :], t_if[0:64, :], 1.0, cblk,
                        ALU.min, ALU.mult)
                    nc.vector.tensor_tensor(cblk, v1[0:64, :], v2[0:64, :],
                                            ALU.add)
                    nc.scalar.activation(th[0:64, :], cblk, AF.Tanh)
                    # h_dec -> B_next bottom (strided image write)
                    nc.vector.scalar_tensor_tensor(
                        Bn[64:128, r0 + 1:r0 + 9, 1:1 + W],
                        t_u[0:64, :].rearrange("p (a b) -> p a b", a=8), 1.0,
                        th[0:64, :].rearrange("p (a b) -> p a b", a=8),
                        ALU.min, ALU.mult)
                    # stage h_dec to DRAM for conv3d: full 130-wide rows
                    # (contiguous; Bn ring cols give dstage cols 0/129 = 0,
                    # corrected by colfix in the conv3d combine)
                    nc.gpsimd.dma_start(
                        dstage[t, :, r0 + 1:r0 + 9, :],
                        Bn[64:128, r0 + 1:r0 + 9, :])

                for t in range(T):
                    # dstage[t] top/bottom pad rows = -t_d/s_d (contiguous,
                    # cheap); col rings stay 0 and are fixed via colfix
                    nc.scalar.dma_start(dstage[t, :, 0, :], zbD[:])
                    nc.scalar.dma_start(dstage[t, :, PW - 1, :], zbD[:])
                    # decoder lags the encoder by 2 superblocks so the
                    # h_enc SBUF->SBUF DMA into B has ~17us to land
                    for k in range(NSB):
                        enc_sb(t, k)
                        if k >= 2:
                            dec_sb(t, k - 2)
                    dec_sb(t, NSB - 2)
                    dec_sb(t, NSB - 1)

            # ---------------- conv3d + sigmoid ----------------
            # ky folded into M (90 = 3ky x 30 outputs): 15 matmuls per
            # 4-row window; ky-shifted partial sums combined on DVE.
            with tc.tile_pool(name="cp", bufs=1) as cp, \
                 tc.tile_pool(name="cpp", bufs=1, space=bass.MemorySpace.PSUM) as cpp:
                D = [cp.tile([128, PW, PW], BF16, name=f"D{q}") for q in range(5)]
                for q in range(5):
                    for j in range(2):
                        eng = (nc.sync, nc.gpsimd, nc.scalar)[(2 * q + j) % 3]
                        eng.dma_start(D[q][64 * j:64 * j + 64, :, :],
                                      dstage[2 * q + j, :, :, :])

                def evict(w):
                    # window w streams D_pad rows w..w+nw-1
                    nw = 2 if w == H else 4
                    py = cpp.tile([96, 512], F32, tag="py", bufs=3, name="py")
                    for q in range(5):
                        for kx in range(3):
                            i = 3 * q + kx
                            nc.tensor.matmul(
                                py[:, 0:128 * nw], w3t[:, i, :],
                                D[q][:, w:w + nw, kx:kx + 128],
                                start=(i == 0), stop=(i == 14))
                    shA = cp.tile([32, 512], F32, tag="shA", bufs=2, name="shA")
                    shB = cp.tile([32, 512], F32, tag="shB", bufs=2, name="shB")
                    shC = cp.tile([32, 512], F32, tag="shC", bufs=2, name="shC")
                    nc.scalar.activation(shA[:, 0:128 * nw], py[0:32, 0:128 * nw],
                                         AF.Copy)
                    nc.scalar.activation(shB[:, 0:128 * nw], py[32:64, 0:128 * nw],
                                         AF.Copy)
                    nc.scalar.activation(shC[:, 0:128 * nw], py[64:96, 0:128 * nw],
                                         AF.Copy)
                    return shA, shB, shC

                prev = evict(0)
                for k in range(NSB * 2):
                    y0 = 4 * k
                    cur, prev = prev, evict(y0 + 4)
                    cA, cB, cC = cur
                    nA, nB, nC = prev
                    u = cp.tile([30, 4, 128], F32, tag="u", bufs=2, name="u")
                    v = cp.tile([30, 4, 128], F32, tag="v", bufs=2, name="v")
                    cA = cA[0:30, :].rearrange("p (a b) -> p a b", b=128)
                    cB = cB[0:30, :].rearrange("p (a b) -> p a b", b=128)
                    cC = cC[0:30, :].rearrange("p (a b) -> p a b", b=128)
                    nB = nB[0:30, :].rearrange("p (a b) -> p a b", b=128)
                    nC = nC[0:30, :].rearrange("p (a b) -> p a b", b=128)
                    nc.vector.tensor_tensor(u[:, 0:3, :], cA[:, 0:3, :],
                                            cB[:, 1:4, :], ALU.add)
                    nc.vector.tensor_tensor(u[:, 3:4, :], cA[:, 3:4, :],
                                            nB[:, 0:1, :], ALU.add)
                    nc.vector.tensor_tensor(v[:, 0:2, :], u[:, 0:2, :],
                                            cC[:, 2:4, :], ALU.add)
                    nc.vector.tensor_tensor(v[:, 2:4, :], u[:, 2:4, :],
                                            nC[:, 0:2, :], ALU.add)
                    # col-ring correction: dstage cols 0/129 hold 0 instead
                    # of -t_d/s_d; add the baked per-(frame,ch) constants
                    for col, c0f in ((0, 0), (127, 3)):
                        segs = [(slice(0, 4), c0f + 1)]
                        if k == 0:
                            segs = [(slice(0, 1), c0f), (slice(1, 4), c0f + 1)]
                        elif k == 2 * NSB - 1:
                            segs = [(slice(0, 3), c0f + 1), (slice(3, 4), c0f + 2)]
                        for rs, ci in segs:
                            nc.vector.tensor_scalar(
                                v[:, rs, col:col + 1], v[:, rs, col:col + 1],
                                colfix[:, ci:ci + 1], None, ALU.add)
                    ty = cp.tile([30, 512], F32, tag="ty", bufs=2, name="ty")
                    nc.scalar.activation(ty[:], v[:].rearrange("p a b -> p (a b)"),
                                         AF.Sigmoid, bias=bout[:], scale=1.0)
                    nc.scalar.dma_start(
                        y_d[:, y0:y0 + 4, :],
                        ty[:].rearrange("p (a b) -> p a b", a=4))

    if do_split:
        split_multi_waits(nc)
    nc.finalize()
    return nc


def _prep(inputs):
    x = np.asarray(inputs["x"], np.float32)
    xpad = np.zeros((8, T, PW, PW, C), np.float32)
    xpad[:, :, 1:1 + H, 1:1 + W, :] = x
    xim = np.empty((8, T, 27, NPIX), BFP)
    for ky in range(3):
        for kx in range(3):
            s = ky * 3 + kx
            v = xpad[:, :, ky:ky + H, kx:kx + W, :]
            xim[:, :, s * 3:s * 3 + 3, :] = (
                v.transpose(0, 1, 4, 2, 3).reshape(8, T, 3, NPIX).astype(BFP))

    enc_Wh = np.asarray(inputs["enc_Wh"], np.float32)
    enc_Wx = np.asarray(inputs["enc_Wx"], np.float32)
    dec_Wx = np.asarray(inputs["dec_Wx"], np.float32)
    dec_Wh = np.asarray(inputs["dec_Wh"], np.float32)
    out_W = np.asarray(inputs["out_W"], np.float32)
    enc_b = np.asarray(inputs["enc_b"], np.float32)
    dec_b = np.asarray(inputs["dec_b"], np.float32)

    # BN affine folds
    s_e = np.asarray(inputs["enc_gamma"], np.float32) / np.sqrt(
        np.asarray(inputs["enc_var"], np.float32) + BN_EPS)
    t_e = np.asarray(inputs["enc_beta"], np.float32) - np.asarray(
        inputs["enc_mean"], np.float32) * s_e
    s_d = np.asarray(inputs["dec_gamma"], np.float32) / np.sqrt(
        np.asarray(inputs["dec_var"], np.float32) + BN_EPS)
    t_d = np.asarray(inputs["dec_beta"], np.float32) - np.asarray(
        inputs["dec_mean"], np.float32) * s_d

    # encoder hidden-conv weights: paired taps (ky,0)+(ky,1), singles (ky,2)
    encp = np.zeros((128, 3, 256), np.float32)
    encs = np.zeros((128, 3, 256), np.float32)
    for ky in range(3):
        encp[0:64, ky, :] = enc_Wh[ky, 0]
        encp[64:128, ky, :] = enc_Wh[ky, 1]
        encs[0:64, ky, :] = enc_Wh[ky, 2]
    encx = np.zeros((128, 256), np.float32)
    encx[0:27, :] = enc_Wx.reshape(27, 256)

    # decoder weights: rows 0:64 dec_Wx scaled by s_e (enc-BN fold),
    # rows 64:128 dec_Wh; gate chunk A permuted to [f|i] so DVE two-input
    # ops get matching base partitions
    perm = np.concatenate([np.arange(64, 128), np.arange(0, 64),
                           np.arange(128, 256)])
    decw = np.zeros((128, 9, 256), np.float32)
    for s in range(9):
        ky, kx = s // 3, s % 3
        decw[0:64, s, :] = (dec_Wx[ky, kx] * s_e[:, None])[:, perm]
        decw[64:128, s, :] = dec_Wh[ky, kx][:, perm]
    # enc-BN shift folded into decoder bias (interior taps; border taps
    # cancelled by the -t_e/s_e ring in B's top half)
    dec_b_f = dec_b + np.einsum("ykcg,c->g", dec_Wx.reshape(3, 3, 64, 256), t_e)

    # conv3d weights scaled by s_d (dec-BN fold); ky folded into M:
    # col 32*ky + (3t+c), 32-row groups for partition-base alignment
    out_W_s = out_W * s_d[None, None, None, :, None]
    w3 = np.zeros((15, 128, 96), np.float32)
    for q in range(5):
        for j in range(2):
            f = 2 * q + j
            for t in range(max(0, f - 1), min(T - 1, f + 1) + 1):
                dt = f - t + 1
                for ky in range(3):
                    for kx in range(3):
                        w3[q * 3 + kx, 64 * j:64 * j + 64,
                           32 * ky + 3 * t:32 * ky + 3 * t + 3] = \
                            out_W_s[dt, ky, kx]
    w3 = w3.transpose(1, 0, 2)  # [128, 15, 96]

    # conv3d bias: out_b + dec-BN shift over valid temporal taps
    out_b = np.asarray(inputs["out_b"], np.float32)
    bout = np.zeros((30,), np.float32)
    for t in range(T):
        acc = out_b.copy()
        for dt in range(3):
            g = t + dt - 1
            if 0 <= g < T:
                acc = acc + np.einsum("ykcg,c->g", out_W[dt], t_d)
        bout[3 * t:3 * t + 3] = acc
    bout = bout.reshape(30, 1)

    aps = np.zeros((128, 4), np.float32)
    aps[0:64, 0] = 0.2 * enc_b[0:64] + 0.5        # enc i
    aps[64:128, 0] = 0.2 * enc_b[64:128] + 0.5    # enc f
    aps[0:64, 1] = enc_b[128:192]                 # enc c~ (tanh bias)
    aps[64:128, 1] = 0.2 * enc_b[192:256] + 0.5   # enc o
    aps[0:64, 2] = 0.2 * dec_b_f[64:128] + 0.5    # dec f (chunk A is [f|i])
    aps[64:128, 2] = 0.2 * dec_b_f[0:64] + 0.5    # dec i
    aps[0:64, 3] = dec_b_f[128:192]               # dec c~
    aps[64:128, 3] = 0.2 * dec_b_f[192:256] + 0.5 # dec o

    # col-ring correction constants: output (t,y,x=0/127) misses the
    # -t_d/s_d ring under tap kx=0/2 for ky reaching interior rows
    colfix = np.zeros((30, 6), np.float32)
    for t in range(T):
        for side, kx in ((0, 0), (3, 2)):
            for ci, kys in ((0, (1, 2)), (1, (0, 1, 2)), (2, (0, 1))):
                acc = np.zeros(3, np.float32)
                for dt in range(3):
                    g = t + dt - 1
                    if 0 <= g < T:
                        for ky in kys:
                            acc -= np.einsum("cg,c->g", out_W[dt, ky, kx], t_d)
                colfix[3 * t:3 * t + 3, side + ci] = acc

    ringb = np.zeros((128, PW), np.float32)
    ringb[0:64, :] = (-t_e / s_e)[:, None]
    ringd = np.tile((-t_d / s_d)[:, None], (1, PW))

    shared = {
        "encp": encp.reshape(128, 3 * 256).astype(BFP),
        "encs": encs.reshape(128, 3 * 256).astype(BFP),
        "encx": encx.astype(BFP),
        "decw": decw.reshape(128, 9 * 256).astype(BFP),
        "w3": w3.reshape(128, 15 * 96).astype(BFP),
        "aps": aps,
        "bout": bout,
        "ringb": ringb.astype(BFP),
        "ringd": ringd.astype(BFP),
        "colfix": colfix,
    }
    return [dict(shared, xim=np.ascontiguousarray(xim[c])) for c in range(8)]


_CACHE = {}


def kernel(**inputs):
    if "nc" not in _CACHE:
        _CACHE["nc"] = _build()
    nc = _CACHE["nc"]
    in_maps = _prep(inputs)
    from concourse.bass_utils import run_bass_kernel_spmd
    res = run_bass_kernel_spmd(nc, in_maps, core_ids=list(range(8)))
    kernel.last_exec_ns = res.exec_time_ns
    kernel.last_res = res
    y = np.stack([
        np.asarray(res.results[c]["y"], np.float32)
        .reshape(T, 3, H, W).transpose(0, 2, 3, 1)
        for c in range(8)
    ])
    return y


# revision 23
# speedup vs baseline: 2.3822x; 1.0327x over previous
"""PredRNN (ConvLSTM enc -> BN -> ConvLSTM dec -> BN -> Conv3D -> sigmoid) on 8 trn2 cores.

Sharding: data-parallel over batch (B=8), one sample per core. Per core:
- Both BNs folded into downstream conv weights; pad-ring constants fix the
  zero-padded borders, so no BN compute on device.
- Encoder hidden conv: h stored twice in the partition dim (plain + shifted
  one column left), so 2 of the 9 taps fuse into one K=128 matmul; all
  encoder matmuls are K=128 (zero-padded weights) to keep the PE HAM clock
  warm (K<128 rhs measured at half clock).
- Decoder: K=128 [h_enc | h_dec] stacked, 9 taps x 2 gate-chunks.
- N=1024 superblocks (8 image rows, 2 PSUM banks per gate-chunk) to amortize
  ACT/DVE fixed overheads; gates evicted with fused hard-sigmoid.
- Conv3D via frame-pair K=128 matmuls with host-baked lhsT, sigmoid eviction.
"""
import sys

sys.path.insert(0, "/opt/trn_rl_repo")
import numpy as np
import ml_dtypes

import concourse.bass as bass
import concourse.tile as tile
from concourse import mybir
from concourse.vector_clock import ScopedClock

BF16 = mybir.dt.bfloat16
F32 = mybir.dt.float32
AF = mybir.ActivationFunctionType
ALU = mybir.AluOpType

T, H, W, F, C = 10, 128, 128, 64, 3
PW = H + 2
NSB = H // 8          # 16 superblocks of 8 rows / 1024 px
NPIX = H * W
BN_EPS = 1e-3
BFP = ml_dtypes.bfloat16


def _patched_drain_and_barrier(self, tick_clock, wait_clock):
    nc = self.nc
    carrier = nc.sync.nop(nofuse=True, hint="drain_waits")
    wait_clock.add_sem_waits(carrier.ins, ScopedClock({None: tick_clock.global_clock}))
    si = carrier.ins.sync_info
    waits = list(si.on_wait) if si is not None else []
    if len(waits) > 1:
        si.on_wait = waits[:1]
        for w in waits[1:]:
            n = nc.sync.nop(nofuse=True, hint="drain_waits")
            n.ins.sync_info = mybir.SyncInfo(on_wait=[w], on_update=[])
    nc.sync.drain()
    nc.all_engine_barrier()
    popped = nc._tile_sem_poison_stack.pop()
    assert popped is self._sem_poison
    nc.clear_and_free_semaphores(list(self.sems.allocated().values()))
    nc.all_engine_barrier()


tile.TileContext._drain_and_barrier = _patched_drain_and_barrier


def split_multi_waits(nc, max_keep=1):
    """Walrus codegen rejects >1 sem wait on compute instructions; hoist
    extras onto same-engine single-wait NOPs inserted just before."""
    n_split = 0
    for fn in nc.m.functions:
        for blk in fn.blocks:
            insts = blk.instructions
            i = 0
            while i < len(insts):
                inst = insts[i]
                si = inst.sync_info
                waits = list(si.on_wait) if si is not None and si.on_wait else []
                if len(waits) > max_keep:
                    for j, w in enumerate(waits[:-max_keep]):
                        nop = mybir.InstNoOp(
                            name=f"{inst.name}_w{j}",
                            engine=inst.engine,
                            sync_info=mybir.SyncInfo(on_wait=[w], on_update=[]),
                            bass_nofuse=True,
                            ins=[],
                            outs=[],
                        )
                        insts.insert(i, nop)
                        i += 1
                    si.on_wait = waits[-max_keep:]
                    n_split += 1
                i += 1
    return n_split


def _build(do_split=True):
    nc = bass.Bass()
    xim_d = nc.dram_tensor("xim", [T, 27, NPIX], BF16, kind="ExternalInput")
    encp_d = nc.dram_tensor("encp", [128, 3 * 256], BF16, kind="ExternalInput")
    encs_d = nc.dram_tensor("encs", [128, 3 * 256], BF16, kind="ExternalInput")
    encx_d = nc.dram_tensor("encx", [128, 256], BF16, kind="ExternalInput")
    decw_d = nc.dram_tensor("decw", [128, 9 * 256], BF16, kind="ExternalInput")
    w3_d = nc.dram_tensor("w3", [128, 15 * 96], BF16, kind="ExternalInput")
    aps_d = nc.dram_tensor("aps", [128, 4], F32, kind="ExternalInput")
    bout_d = nc.dram_tensor("bout", [30, 1], F32, kind="ExternalInput")
    ringb_d = nc.dram_tensor("ringb", [128, PW], BF16, kind="ExternalInput")
    ringd_d = nc.dram_tensor("ringd", [64, PW], BF16, kind="ExternalInput")
    colfix_d = nc.dram_tensor("colfix", [30, 6], F32, kind="ExternalInput")
    dstage = nc.dram_tensor("dstage", [T, F, PW, PW], BF16, kind="Internal")
    y_d = nc.dram_tensor("y", [30, H, W], F32, kind="ExternalOutput")

    with tile.TileContext(nc) as tc:
        with tc.tile_pool(name="wp", bufs=1) as wp:
            encp = wp.tile([128, 3, 256], BF16)
            encs = wp.tile([128, 3, 256], BF16)
            encx = wp.tile([128, 256], BF16)
            decw = wp.tile([128, 9, 256], BF16)
            w3t = wp.tile([128, 15, 96], BF16)
            aps = wp.tile([128, 4], F32)
            bout = wp.tile([30, 1], F32)
            zbB = wp.tile([128, PW], BF16)
            zbD = wp.tile([64, PW], BF16)
            colfix = wp.tile([30, 6], F32)
            nc.sync.dma_start(encp[:], encp_d[:].rearrange("p (s g) -> p s g", s=3))
            nc.sync.dma_start(encs[:], encs_d[:].rearrange("p (s g) -> p s g", s=3))
            nc.sync.dma_start(encx[:], encx_d[:])
            nc.sync.dma_start(decw[:], decw_d[:].rearrange("p (s g) -> p s g", s=9))
            nc.sync.dma_start(w3t[:], w3_d[:].rearrange("p (s g) -> p s g", s=15))
            nc.sync.dma_start(aps[:], aps_d[:])
            nc.sync.dma_start(bout[:], bout_d[:])
            nc.sync.dma_start(zbB[:], ringb_d[:])
            nc.sync.dma_start(zbD[:], ringd_d[:])
            nc.sync.dma_start(colfix[:], colfix_d[:])

            with tc.tile_pool(name="sp", bufs=1) as sp, \
                 tc.tile_pool(name="tp", bufs=1) as tp, \
                 tc.tile_pool(name="pp", bufs=1, space=bass.MemorySpace.PSUM) as pp:
                E0 = sp.tile([128, PW, PW], BF16)
                E1 = sp.tile([128, PW, PW], BF16)
                B0 = sp.tile([128, PW, PW], BF16)
                B1 = sp.tile([128, PW, PW], BF16)
                CC = sp.tile([128, NPIX], BF16)
                # init xs staging buffers to zero first (rows 27:128 stay
                # zero, multiplied by zero weight rows -> avoids NaN garbage)
                for _ in range(2):
                    xs0 = tp.tile([128, 1024], BF16, tag="xs", bufs=2, name="xs0")
                    nc.vector.memset(xs0[:], 0.0)
                nc.vector.memset(E0[:], 0.0)   # h_enc(-1) = 0
                nc.vector.memset(CC[:], 0.0)   # c(-1) = 0
                # E1 only needs its pad ring zeroed (interior fully written
                # at t=0); copy2 tail cols 128:130 also never written
                nc.vector.memset(E1[:, 0, :], 0.0)
                nc.vector.memset(E1[:, PW - 1, :], 0.0)
                nc.vector.memset(E1[:, :, 0:1], 0.0)
                nc.vector.memset(E1[:, :, PW - 2:PW], 0.0)
                # B0 bottom half: h_dec(-1) = 0; B1 interior is written
                # before it is read, so only rings are needed
                nc.vector.memset(B0[64:128, :, :], 0.0)
                for B in (B0, B1):
                    # top ring = -t_e/s_e (rows 0:64), bottom ring = 0
                    nc.gpsimd.tensor_copy(B[:, 0, :], zbB[:])
                    nc.gpsimd.tensor_copy(B[:, PW - 1, :], zbB[:])
                    nc.gpsimd.tensor_copy(
                        B[:, :, 0:1], zbB[:].rearrange("p (w o) -> p w o", o=1))
                    nc.gpsimd.tensor_copy(
                        B[:, :, PW - 1:PW], zbB[:].rearrange("p (w o) -> p w o", o=1))
                Es = [E0, E1]
                Bs = [B0, B1]

                def enc_sb(t, k):
                    Ep, En = Es[t % 2], Es[(t + 1) % 2]
                    Bc = Bs[t % 2]
                    r0, c0 = 8 * k, 1024 * k
                    xs = tp.tile([128, 1024], BF16, tag="xs", bufs=2)
                    nc.sync.dma_start(xs[0:27, :], xim_d[t, :, c0:c0 + 1024])
                    pA = pp.tile([128, 1024], F32, tag="pA", bufs=2)
                    pB = pp.tile([128, 1024], F32, tag="pB", bufs=2)
                    for ch, ps in ((0, pA), (1, pB)):
                        cw = slice(128 * ch, 128 * ch + 128)
                        for hf in range(2):
                            rh = r0 + 4 * hf
                            out = ps[:, 512 * hf:512 * hf + 512]
                            for ky in range(3):  # paired taps (ky,0)+(ky,1)
                                nc.tensor.matmul(
                                    out, encp[:, ky, cw],
                                    Ep[:, rh + ky:rh + ky + 4, 0:128],
                                    start=(ky == 0), stop=False)
                            for ky in range(3):  # single taps (ky,2)
                                nc.tensor.matmul(
                                    out, encs[:, ky, cw],
                                    Ep[:, rh + ky:rh + ky + 4, 2:130],
                                    start=False, stop=False)
                            nc.tensor.matmul(
                                out, encx[:, cw], xs[:, 512 * hf:512 * hf + 512],
                                start=False, stop=True)
                    t_if = tp.tile([128, 1024], F32, tag="t_if", bufs=2, name="t_if")
                    t_u = tp.tile([128, 1024], BF16, tag="t_u", bufs=2, name="t_u")
                    v1 = tp.tile([128, 1024], BF16, tag="t_v", bufs=3, name="v1")
                    v2 = tp.tile([128, 1024], BF16, tag="t_v", bufs=3, name="v2")
                    th = tp.tile([128, 1024], BF16, tag="t_v", bufs=3, name="th")
                    cblk = CC[64:128, c0:c0 + 1024]
                    nc.scalar.activation(t_if[:], pA[:], AF.Relu,
                                         bias=aps[:, 0:1], scale=0.2)
                    nc.scalar.activation(t_u[0:64, :], pB[0:64, :], AF.Tanh,
                                         bias=aps[0:64, 1:2], scale=1.0)
                    nc.scalar.activation(t_u[64:128, :], pB[64:128, :], AF.Relu,
                                         bias=aps[64:128, 1:2], scale=0.2)
                    nc.vector.scalar_tensor_tensor(
                        v1[64:128, :], t_if[0:64, :], 1.0, t_u[0:64, :],
                        ALU.min, ALU.mult)
                    nc.vector.scalar_tensor_tensor(
                        v2[64:128, :], t_if[64:128, :], 1.0, cblk,
                        ALU.min, ALU.mult)
                    nc.vector.tensor_tensor(cblk, v1[64:128, :], v2[64:128, :],
                                            ALU.add)
                    nc.scalar.activation(th[64:128, :], cblk, AF.Tanh)
                    # h -> E_next copy1 (strided image write, fused o*tanh(c))
                    nc.vector.scalar_tensor_tensor(
                        En[0:64, r0 + 1:r0 + 9, 1:1 + W],
                        t_u[64:128, :].rearrange("p (a b) -> p a b", a=8), 1.0,
                        th[64:128, :].rearrange("p (a b) -> p a b", a=8),
                        ALU.min, ALU.mult)
                    # copy2: shifted one column left (for paired taps)
                    nc.gpsimd.tensor_copy(
                        En[64:128, r0 + 1:r0 + 9, 0:W],
                        En[0:64, r0 + 1:r0 + 9, 1:1 + W])
                    # B_cur top: h_enc for the decoder (SBUF->SBUF DMA)
                    nc.sync.dma_start(
                        Bc[0:64, r0 + 1:r0 + 9, 1:1 + W],
                        En[0:64, r0 + 1:r0 + 9, 1:1 + W])

                def dec_sb(t, k):
                    Bc, Bn = Bs[t % 2], Bs[(t + 1) % 2]
                    r0, c0 = 8 * k, 1024 * k
                    pA = pp.tile([128, 1024], F32, tag="pA", bufs=2)
                    pB = pp.tile([128, 1024], F32, tag="pB", bufs=2)
                    for ch, ps in ((0, pA), (1, pB)):
                        cw = slice(128 * ch, 128 * ch + 128)
                        for hf in range(2):
                            rh = r0 + 4 * hf
                            out = ps[:, 512 * hf:512 * hf + 512]
                            for s in range(9):
                                ky, kx = s // 3, s % 3
                                nc.tensor.matmul(
                                    out, decw[:, s, cw],
                                    Bc[:, rh + ky:rh + ky + 4, kx:kx + 128],
                                    start=(s == 0), stop=(s == 8))
                    t_if = tp.tile([128, 1024], F32, tag="t_if", bufs=2, name="t_if")
                    t_u = tp.tile([128, 1024], BF16, tag="t_u", bufs=2, name="t_u")
                    v1 = tp.tile([128, 1024], BF16, tag="t_v", bufs=3, name="v1")
                    v2 = tp.tile([128, 1024], BF16, tag="t_v", bufs=3, name="v2")
                    th = tp.tile([128, 1024], BF16, tag="t_v", bufs=3, name="th")
                    cblk = CC[0:64, c0:c0 + 1024]
                    # dec chunk A is [f|i] (host-permuted); tanh/relu ACT
                    # outputs criss-cross partitions so DVE input bases match
                    nc.scalar.activation(t_if[:], pA[:], AF.Relu,
                                         bias=aps[:, 2:3], scale=0.2)
                    nc.scalar.activation(t_u[64:128, :], pB[0:64, :], AF.Tanh,
                                         bias=aps[0:64, 3:4], scale=1.0)
                    nc.scalar.activation(t_u[0:64, :], pB[64:128, :], AF.Relu,
                                         bias=aps[64:128, 3:4], scale=0.2)
                    nc.vector.scalar_tensor_tensor(
                        v1[0:64, :], t_if[64:128, :], 1.0, t_u[64:128, :],
                        ALU.min, ALU.mult)
                    nc.vector.scalar_tensor_tensor(
                        v2[0:64, :], t_if[0:64, :], 1.0, cblk,
                        ALU.min, ALU.mult)
                    nc.vector.tensor_tensor(cblk, v1[0:64, :], v2[0:64, :],
                                            ALU.add)
                    nc.scalar.activation(th[0:64, :], cblk, AF.Tanh)
                    # h_dec -> B_next bottom (strided image write)
                    nc.vector.scalar_tensor_tensor(
                        Bn[64:128, r0 + 1:r0 + 9, 1:1 + W],
                        t_u[0:64, :].rearrange("p (a b) -> p a b", a=8), 1.0,
                        th[0:64, :].rearrange("p (a b) -> p a b", a=8),
                        ALU.min, ALU.mult)
                    # stage h_dec to DRAM for conv3d: full 130-wide rows
                    # (contiguous; Bn ring cols give dstage cols 0/129 = 0,
                    # corrected by colfix in the conv3d combine)
                    nc.gpsimd.dma_start(
                        dstage[t, :, r0 + 1:r0 + 9, :],
                        Bn[64:128, r0 + 1:r0 + 9, :])

                for t in range(T):
                    # dstage[t] top/bottom pad rows = -t_d/s_d (contiguous,
                    # cheap); col rings stay 0 and are fixed via colfix
                    nc.scalar.dma_start(dstage[t, :, 0, :], zbD[:])
                    nc.scalar.dma_start(dstage[t, :, PW - 1, :], zbD[:])
                    # decoder lags the encoder by 2 superblocks so the
                    # h_enc SBUF->SBUF DMA into B has ~17us to land
                    for k in range(NSB):
                        enc_sb(t, k)
                        if k >= 3:
                            dec_sb(t, k - 3)
                    for k in (NSB - 3, NSB - 2, NSB - 1):
                        dec_sb(t, k)

            # ---------------- conv3d + sigmoid ----------------
            # ky folded into M (90 = 3ky x 30 outputs): 15 matmuls per
            # 4-row window; ky-shifted partial sums combined on DVE.
            with tc.tile_pool(name="cp", bufs=1) as cp, \
                 tc.tile_pool(name="cpp", bufs=1, space=bass.MemorySpace.PSUM) as cpp:
                D = [cp.tile([128, PW, PW], BF16, name=f"D{q}") for q in range(5)]
                # load top row-halves of all frames first so early windows
                # can start while the bottom halves stream in
                for rr in (slice(0, 66), slice(66, PW)):
                    for q in range(5):
                        for j in range(2):
                            eng = (nc.sync, nc.gpsimd, nc.scalar)[(2 * q + j) % 3]
                            eng.dma_start(D[q][64 * j:64 * j + 64, rr, :],
                                          dstage[2 * q + j, :, rr, :])

                def evict(w):
                    # window w streams D_pad rows w..w+nw-1
                    nw = 2 if w == H else 4
                    py = cpp.tile([96, 512], F32, tag="py", bufs=3, name="py")
                    for q in range(5):
                        for kx in range(3):
                            i = 3 * q + kx
                            nc.tensor.matmul(
                                py[:, 0:128 * nw], w3t[:, i, :],
                                D[q][:, w:w + nw, kx:kx + 128],
                                start=(i == 0), stop=(i == 14))
                    shA = cp.tile([32, 512], F32, tag="shA", bufs=2, name="shA")
                    shB = cp.tile([32, 512], F32, tag="shB", bufs=2, name="shB")
                    shC = cp.tile([32, 512], F32, tag="shC", bufs=2, name="shC")
                    nc.scalar.activation(shA[:, 0:128 * nw], py[0:32, 0:128 * nw],
                                         AF.Copy)
                    nc.scalar.activation(shB[:, 0:128 * nw], py[32:64, 0:128 * nw],
                                         AF.Copy)
                    nc.scalar.activation(shC[:, 0:128 * nw], py[64:96, 0:128 * nw],
                                         AF.Copy)
                    return shA, shB, shC

                prev = evict(0)
                for k in range(NSB * 2):
                    y0 = 4 * k
                    cur, prev = prev, evict(y0 + 4)
                    cA, cB, cC = cur
                    nA, nB, nC = prev
                    u = cp.tile([30, 4, 128], F32, tag="u", bufs=2, name="u")
                    v = cp.tile([30, 4, 128], F32, tag="v", bufs=2, name="v")
                    cA = cA[0:30, :].rearrange("p (a b) -> p a b", b=128)
                    cB = cB[0:30, :].rearrange("p (a b) -> p a b", b=128)
                    cC = cC[0:30, :].rearrange("p (a b) -> p a b", b=128)
                    nB = nB[0:30, :].rearrange("p (a b) -> p a b", b=128)
                    nC = nC[0:30, :].rearrange("p (a b) -> p a b", b=128)
                    nc.vector.tensor_tensor(u[:, 0:3, :], cA[:, 0:3, :],
                                            cB[:, 1:4, :], ALU.add)
                    nc.vector.tensor_tensor(u[:, 3:4, :], cA[:, 3:4, :],
                                            nB[:, 0:1, :], ALU.add)
                    nc.vector.tensor_tensor(v[:, 0:2, :], u[:, 0:2, :],
                                            cC[:, 2:4, :], ALU.add)
                    nc.vector.tensor_tensor(v[:, 2:4, :], u[:, 2:4, :],
                                            nC[:, 0:2, :], ALU.add)
                    # col-ring correction: dstage cols 0/129 hold 0 instead
                    # of -t_d/s_d; add the baked per-(frame,ch) constants
                    for col, c0f in ((0, 0), (127, 3)):
                        segs = [(slice(0, 4), c0f + 1)]
                        if k == 0:
                            segs = [(slice(0, 1), c0f), (slice(1, 4), c0f + 1)]
                        elif k == 2 * NSB - 1:
                            segs = [(slice(0, 3), c0f + 1), (slice(3, 4), c0f + 2)]
                        for rs, ci in segs:
                            nc.vector.tensor_scalar(
                                v[:, rs, col:col + 1], v[:, rs, col:col + 1],
                                colfix[:, ci:ci + 1], None, ALU.add)
                    ty = cp.tile([30, 512], F32, tag="ty", bufs=2, name="ty")
                    nc.scalar.activation(ty[:], v[:].rearrange("p a b -> p (a b)"),
                                         AF.Sigmoid, bias=bout[:], scale=1.0)
                    nc.scalar.dma_start(
                        y_d[:, y0:y0 + 4, :],
                        ty[:].rearrange("p (a b) -> p a b", a=4))

    if do_split:
        split_multi_waits(nc)
    nc.finalize()
    return nc


def _prep(inputs):
    x = np.asarray(inputs["x"], np.float32)
    xpad = np.zeros((8, T, PW, PW, C), np.float32)
    xpad[:, :, 1:1 + H, 1:1 + W, :] = x
    xim = np.empty((8, T, 27, NPIX), BFP)
    for ky in range(3):
        for kx in range(3):
            s = ky * 3 + kx
            v = xpad[:, :, ky:ky + H, kx:kx + W, :]
            xim[:, :, s * 3:s * 3 + 3, :] = (
                v.transpose(0, 1, 4, 2, 3).reshape(8, T, 3, NPIX).astype(BFP))

    enc_Wh = np.asarray(inputs["enc_Wh"], np.float32)
    enc_Wx = np.asarray(inputs["enc_Wx"], np.float32)
    dec_Wx = np.asarray(inputs["dec_Wx"], np.float32)
    dec_Wh = np.asarray(inputs["dec_Wh"], np.float32)
    out_W = np.asarray(inputs["out_W"], np.float32)
    enc_b = np.asarray(inputs["enc_b"], np.float32)
    dec_b = np.asarray(inputs["dec_b"], np.float32)

    # BN affine folds
    s_e = np.asarray(inputs["enc_gamma"], np.float32) / np.sqrt(
        np.asarray(inputs["enc_var"], np.float32) + BN_EPS)
    t_e = np.asarray(inputs["enc_beta"], np.float32) - np.asarray(
        inputs["enc_mean"], np.float32) * s_e
    s_d = np.asarray(inputs["dec_gamma"], np.float32) / np.sqrt(
        np.asarray(inputs["dec_var"], np.float32) + BN_EPS)
    t_d = np.asarray(inputs["dec_beta"], np.float32) - np.asarray(
        inputs["dec_mean"], np.float32) * s_d

    # encoder hidden-conv weights: paired taps (ky,0)+(ky,1), singles (ky,2)
    encp = np.zeros((128, 3, 256), np.float32)
    encs = np.zeros((128, 3, 256), np.float32)
    for ky in range(3):
        encp[0:64, ky, :] = enc_Wh[ky, 0]
        encp[64:128, ky, :] = enc_Wh[ky, 1]
        encs[0:64, ky, :] = enc_Wh[ky, 2]
    encx = np.zeros((128, 256), np.float32)
    encx[0:27, :] = enc_Wx.reshape(27, 256)

    # decoder weights: rows 0:64 dec_Wx scaled by s_e (enc-BN fold),
    # rows 64:128 dec_Wh; gate chunk A permuted to [f|i] so DVE two-input
    # ops get matching base partitions
    perm = np.concatenate([np.arange(64, 128), np.arange(0, 64),
                           np.arange(128, 256)])
    decw = np.zeros((128, 9, 256), np.float32)
    for s in range(9):
        ky, kx = s // 3, s % 3
        decw[0:64, s, :] = (dec_Wx[ky, kx] * s_e[:, None])[:, perm]
        decw[64:128, s, :] = dec_Wh[ky, kx][:, perm]
    # enc-BN shift folded into decoder bias (interior taps; border taps
    # cancelled by the -t_e/s_e ring in B's top half)
    dec_b_f = dec_b + np.einsum("ykcg,c->g", dec_Wx.reshape(3, 3, 64, 256), t_e)

    # conv3d weights scaled by s_d (dec-BN fold); ky folded into M:
    # col 32*ky + (3t+c), 32-row groups for partition-base alignment
    out_W_s = out_W * s_d[None, None, None, :, None]
    w3 = np.zeros((15, 128, 96), np.float32)
    for q in range(5):
        for j in range(2):
            f = 2 * q + j
            for t in range(max(0, f - 1), min(T - 1, f + 1) + 1):
                dt = f - t + 1
                for ky in range(3):
                    for kx in range(3):
                        w3[q * 3 + kx, 64 * j:64 * j + 64,
                           32 * ky + 3 * t:32 * ky + 3 * t + 3] = \
                            out_W_s[dt, ky, kx]
    w3 = w3.transpose(1, 0, 2)  # [128, 15, 96]

    # conv3d bias: out_b + dec-BN shift over valid temporal taps
    out_b = np.asarray(inputs["out_b"], np.float32)
    bout = np.zeros((30,), np.float32)
    for t in range(T):
        acc = out_b.copy()
        for dt in range(3):
            g = t + dt - 1
            if 0 <= g < T:
                acc = acc + np.einsum("ykcg,c->g", out_W[dt], t_d)
        bout[3 * t:3 * t + 3] = acc
    bout = bout.reshape(30, 1)

    aps = np.zeros((128, 4), np.float32)
    aps[0:64, 0] = 0.2 * enc_b[0:64] + 0.5        # enc i
    aps[64:128, 0] = 0.2 * enc_b[64:128] + 0.5    # enc f
    aps[0:64, 1] = enc_b[128:192]                 # enc c~ (tanh bias)
    aps[64:128, 1] = 0.2 * enc_b[192:256] + 0.5   # enc o
    aps[0:64, 2] = 0.2 * dec_b_f[64:128] + 0.5    # dec f (chunk A is [f|i])
    aps[64:128, 2] = 0.2 * dec_b_f[0:64] + 0.5    # dec i
    aps[0:64, 3] = dec_b_f[128:192]               # dec c~
    aps[64:128, 3] = 0.2 * dec_b_f[192:256] + 0.5 # dec o

    # col-ring correction constants: output (t,y,x=0/127) misses the
    # -t_d/s_d ring under tap kx=0/2 for ky reaching interior rows
    colfix = np.zeros((30, 6), np.float32)
    for t in range(T):
        for side, kx in ((0, 0), (3, 2)):
            for ci, kys in ((0, (1, 2)), (1, (0, 1, 2)), (2, (0, 1))):
                acc = np.zeros(3, np.float32)
                for dt in range(3):
                    g = t + dt - 1
                    if 0 <= g < T:
                        for ky in kys:
                            acc -= np.einsum("cg,c->g", out_W[dt, ky, kx], t_d)
                colfix[3 * t:3 * t + 3, side + ci] = acc

    ringb = np.zeros((128, PW), np.float32)
    ringb[0:64, :] = (-t_e / s_e)[:, None]
    ringd = np.tile((-t_d / s_d)[:, None], (1, PW))

    shared = {
        "encp": encp.reshape(128, 3 * 256).astype(BFP),
        "encs": encs.reshape(128, 3 * 256).astype(BFP),
        "encx": encx.astype(BFP),
        "decw": decw.reshape(128, 9 * 256).astype(BFP),
        "w3": w3.reshape(128, 15 * 96).astype(BFP),
        "aps": aps,
        "bout": bout,
        "ringb": ringb.astype(BFP),
        "ringd": ringd.astype(BFP),
        "colfix": colfix,
    }
    return [dict(shared, xim=np.ascontiguousarray(xim[c])) for c in range(8)]


_CACHE = {}


def kernel(**inputs):
    if "nc" not in _CACHE:
        _CACHE["nc"] = _build()
    nc = _CACHE["nc"]
    in_maps = _prep(inputs)
    from concourse.bass_utils import run_bass_kernel_spmd
    res = run_bass_kernel_spmd(nc, in_maps, core_ids=list(range(8)))
    kernel.last_exec_ns = res.exec_time_ns
    kernel.last_res = res
    y = np.stack([
        np.asarray(res.results[c]["y"], np.float32)
        .reshape(T, 3, H, W).transpose(0, 2, 3, 1)
        for c in range(8)
    ])
    return y


# revision 24
# speedup vs baseline: 2.4323x; 1.0211x over previous
"""PredRNN (ConvLSTM enc -> BN -> ConvLSTM dec -> BN -> Conv3D -> sigmoid) on 8 trn2 cores.

Sharding: data-parallel over batch (B=8), one sample per core. Per core:
- Both BNs folded into downstream conv weights; pad-ring constants fix the
  zero-padded borders, so no BN compute on device.
- Encoder hidden conv: h stored twice in the partition dim (plain + shifted
  one column left), so 2 of the 9 taps fuse into one K=128 matmul; all
  encoder matmuls are K=128 (zero-padded weights) to keep the PE HAM clock
  warm (K<128 rhs measured at half clock).
- Decoder: K=128 [h_enc | h_dec] stacked, 9 taps x 2 gate-chunks.
- N=1024 superblocks (8 image rows, 2 PSUM banks per gate-chunk) to amortize
  ACT/DVE fixed overheads; gates evicted with fused hard-sigmoid.
- Conv3D via frame-pair K=128 matmuls with host-baked lhsT, sigmoid eviction.
"""
import sys

sys.path.insert(0, "/opt/trn_rl_repo")
import numpy as np
import ml_dtypes

import concourse.bass as bass
import concourse.tile as tile
from concourse import mybir
from concourse.vector_clock import ScopedClock

BF16 = mybir.dt.bfloat16
F32 = mybir.dt.float32
AF = mybir.ActivationFunctionType
ALU = mybir.AluOpType

T, H, W, F, C = 10, 128, 128, 64, 3
PW = H + 2
NSB = H // 8          # 16 superblocks of 8 rows / 1024 px
NPIX = H * W
BN_EPS = 1e-3
BFP = ml_dtypes.bfloat16


def _patched_drain_and_barrier(self, tick_clock, wait_clock):
    nc = self.nc
    carrier = nc.sync.nop(nofuse=True, hint="drain_waits")
    wait_clock.add_sem_waits(carrier.ins, ScopedClock({None: tick_clock.global_clock}))
    si = carrier.ins.sync_info
    waits = list(si.on_wait) if si is not None else []
    if len(waits) > 1:
        si.on_wait = waits[:1]
        for w in waits[1:]:
            n = nc.sync.nop(nofuse=True, hint="drain_waits")
            n.ins.sync_info = mybir.SyncInfo(on_wait=[w], on_update=[])
    nc.sync.drain()
    nc.all_engine_barrier()
    popped = nc._tile_sem_poison_stack.pop()
    assert popped is self._sem_poison
    nc.clear_and_free_semaphores(list(self.sems.allocated().values()))
    nc.all_engine_barrier()


tile.TileContext._drain_and_barrier = _patched_drain_and_barrier


def split_multi_waits(nc, max_keep=1):
    """Walrus codegen rejects >1 sem wait on compute instructions; hoist
    extras onto same-engine single-wait NOPs inserted just before."""
    n_split = 0
    for fn in nc.m.functions:
        for blk in fn.blocks:
            insts = blk.instructions
            i = 0
            while i < len(insts):
                inst = insts[i]
                si = inst.sync_info
                waits = list(si.on_wait) if si is not None and si.on_wait else []
                if len(waits) > max_keep:
                    for j, w in enumerate(waits[:-max_keep]):
                        nop = mybir.InstNoOp(
                            name=f"{inst.name}_w{j}",
                            engine=inst.engine,
                            sync_info=mybir.SyncInfo(on_wait=[w], on_update=[]),
                            bass_nofuse=True,
                            ins=[],
                            outs=[],
                        )
                        insts.insert(i, nop)
                        i += 1
                    si.on_wait = waits[-max_keep:]
                    n_split += 1
                i += 1
    return n_split


def _build(do_split=True):
    nc = bass.Bass()
    xim_d = nc.dram_tensor("xim", [T, 27, NPIX], BF16, kind="ExternalInput")
    encp_d = nc.dram_tensor("encp", [128, 3 * 256], BF16, kind="ExternalInput")
    encs_d = nc.dram_tensor("encs", [128, 3 * 256], BF16, kind="ExternalInput")
    encx_d = nc.dram_tensor("encx", [128, 256], BF16, kind="ExternalInput")
    decw_d = nc.dram_tensor("decw", [128, 9 * 256], BF16, kind="ExternalInput")
    w3_d = nc.dram_tensor("w3", [128, 15 * 96], BF16, kind="ExternalInput")
    aps_d = nc.dram_tensor("aps", [128, 4], F32, kind="ExternalInput")
    bout_d = nc.dram_tensor("bout", [30, 1], F32, kind="ExternalInput")
    ringb_d = nc.dram_tensor("ringb", [128, PW], BF16, kind="ExternalInput")
    ringd_d = nc.dram_tensor("ringd", [64, PW], BF16, kind="ExternalInput")
    colfix_d = nc.dram_tensor("colfix", [30, 6], F32, kind="ExternalInput")
    dstage = nc.dram_tensor("dstage", [T, F, PW, PW], BF16, kind="Internal")
    y_d = nc.dram_tensor("y", [30, H, W], F32, kind="ExternalOutput")

    with tile.TileContext(nc) as tc:
        with tc.tile_pool(name="wp", bufs=1) as wp:
            encp = wp.tile([128, 3, 256], BF16)
            encs = wp.tile([128, 3, 256], BF16)
            encx = wp.tile([128, 256], BF16)
            decw = wp.tile([128, 9, 256], BF16)
            w3t = wp.tile([128, 15, 96], BF16)
            aps = wp.tile([128, 4], F32)
            bout = wp.tile([30, 1], F32)
            zbB = wp.tile([128, PW], BF16)
            zbD = wp.tile([64, PW], BF16)
            colfix = wp.tile([30, 6], F32)
            nc.sync.dma_start(encp[:], encp_d[:].rearrange("p (s g) -> p s g", s=3))
            nc.sync.dma_start(encs[:], encs_d[:].rearrange("p (s g) -> p s g", s=3))
            nc.sync.dma_start(encx[:], encx_d[:])
            nc.sync.dma_start(decw[:], decw_d[:].rearrange("p (s g) -> p s g", s=9))
            nc.sync.dma_start(w3t[:], w3_d[:].rearrange("p (s g) -> p s g", s=15))
            nc.sync.dma_start(aps[:], aps_d[:])
            nc.sync.dma_start(bout[:], bout_d[:])
            nc.sync.dma_start(zbB[:], ringb_d[:])
            nc.sync.dma_start(zbD[:], ringd_d[:])
            nc.sync.dma_start(colfix[:], colfix_d[:])

            with tc.tile_pool(name="sp", bufs=1) as sp, \
                 tc.tile_pool(name="tp", bufs=1) as tp, \
                 tc.tile_pool(name="pp", bufs=1, space=bass.MemorySpace.PSUM) as pp:
                E0 = sp.tile([128, PW, PW], BF16)
                E1 = sp.tile([128, PW, PW], BF16)
                B0 = sp.tile([128, PW, PW], BF16)
                B1 = sp.tile([128, PW, PW], BF16)
                CC = sp.tile([128, NPIX], BF16)
                # init xs staging buffers to zero first (rows 27:128 stay
                # zero, multiplied by zero weight rows -> avoids NaN garbage)
                for _ in range(2):
                    xs0 = tp.tile([128, 1024], BF16, tag="xs", bufs=2, name="xs0")
                    nc.vector.memset(xs0[:], 0.0)
                nc.vector.memset(CC[:], 0.0)   # c(-1) = 0
                # E0/E1 only need their pad rings zeroed: t=0 skips the
                # hidden-conv matmuls entirely (h(-1)=0), and each buffer's
                # interior is fully written before it is first read
                for E in (E0, E1):
                    nc.vector.memset(E[:, 0, :], 0.0)
                    nc.vector.memset(E[:, PW - 1, :], 0.0)
                    nc.vector.memset(E[:, :, 0:1], 0.0)
                    nc.vector.memset(E[:, :, PW - 2:PW], 0.0)
                # B0 bottom half: h_dec(-1) = 0; B1 interior is written
                # before it is read, so only rings are needed
                nc.vector.memset(B0[64:128, :, :], 0.0)
                for B in (B0, B1):
                    # top ring = -t_e/s_e (rows 0:64), bottom ring = 0
                    nc.gpsimd.tensor_copy(B[:, 0, :], zbB[:])
                    nc.gpsimd.tensor_copy(B[:, PW - 1, :], zbB[:])
                    nc.gpsimd.tensor_copy(
                        B[:, :, 0:1], zbB[:].rearrange("p (w o) -> p w o", o=1))
                    nc.gpsimd.tensor_copy(
                        B[:, :, PW - 1:PW], zbB[:].rearrange("p (w o) -> p w o", o=1))
                Es = [E0, E1]
                Bs = [B0, B1]

                def enc_sb(t, k):
                    Ep, En = Es[t % 2], Es[(t + 1) % 2]
                    Bc = Bs[t % 2]
                    r0, c0 = 8 * k, 1024 * k
                    xs = tp.tile([128, 1024], BF16, tag="xs", bufs=2)
                    nc.sync.dma_start(xs[0:27, :], xim_d[t, :, c0:c0 + 1024])
                    pA = pp.tile([128, 1024], F32, tag="pA", bufs=2)
                    pB = pp.tile([128, 1024], F32, tag="pB", bufs=2)
                    for ch, ps in ((0, pA), (1, pB)):
                        cw = slice(128 * ch, 128 * ch + 128)
                        for hf in range(2):
                            rh = r0 + 4 * hf
                            out = ps[:, 512 * hf:512 * hf + 512]
                            if t > 0:  # h(-1)=0: skip hidden taps at t=0
                                for ky in range(3):  # paired taps (ky,0)+(ky,1)
                                    nc.tensor.matmul(
                                        out, encp[:, ky, cw],
                                        Ep[:, rh + ky:rh + ky + 4, 0:128],
                                        start=(ky == 0), stop=False)
                                for ky in range(3):  # single taps (ky,2)
                                    nc.tensor.matmul(
                                        out, encs[:, ky, cw],
                                        Ep[:, rh + ky:rh + ky + 4, 2:130],
                                        start=False, stop=False)
                            nc.tensor.matmul(
                                out, encx[:, cw], xs[:, 512 * hf:512 * hf + 512],
                                start=(t == 0), stop=True)
                    t_if = tp.tile([128, 1024], F32, tag="t_if", bufs=2, name="t_if")
                    t_u = tp.tile([128, 1024], BF16, tag="t_u", bufs=2, name="t_u")
                    v1 = tp.tile([128, 1024], BF16, tag="t_v", bufs=3, name="v1")
                    v2 = tp.tile([128, 1024], BF16, tag="t_v", bufs=3, name="v2")
                    th = tp.tile([128, 1024], BF16, tag="t_v", bufs=3, name="th")
                    cblk = CC[64:128, c0:c0 + 1024]
                    nc.scalar.activation(t_if[:], pA[:], AF.Relu,
                                         bias=aps[:, 0:1], scale=0.2)
                    nc.scalar.activation(t_u[0:64, :], pB[0:64, :], AF.Tanh,
                                         bias=aps[0:64, 1:2], scale=1.0)
                    nc.scalar.activation(t_u[64:128, :], pB[64:128, :], AF.Relu,
                                         bias=aps[64:128, 1:2], scale=0.2)
                    nc.vector.scalar_tensor_tensor(
                        v1[64:128, :], t_if[0:64, :], 1.0, t_u[0:64, :],
                        ALU.min, ALU.mult)
                    nc.vector.scalar_tensor_tensor(
                        v2[64:128, :], t_if[64:128, :], 1.0, cblk,
                        ALU.min, ALU.mult)
                    nc.vector.tensor_tensor(cblk, v1[64:128, :], v2[64:128, :],
                                            ALU.add)
                    nc.scalar.activation(th[64:128, :], cblk, AF.Tanh)
                    # h -> E_next copy1 (strided image write, fused o*tanh(c))
                    nc.vector.scalar_tensor_tensor(
                        En[0:64, r0 + 1:r0 + 9, 1:1 + W],
                        t_u[64:128, :].rearrange("p (a b) -> p a b", a=8), 1.0,
                        th[64:128, :].rearrange("p (a b) -> p a b", a=8),
                        ALU.min, ALU.mult)
                    if t < T - 1:
                        # copy2: shifted one column left (for paired taps)
                        nc.gpsimd.tensor_copy(
                            En[64:128, r0 + 1:r0 + 9, 0:W],
                            En[0:64, r0 + 1:r0 + 9, 1:1 + W])
                    # B_cur top: h_enc for the decoder (SBUF->SBUF DMA)
                    nc.sync.dma_start(
                        Bc[0:64, r0 + 1:r0 + 9, 1:1 + W],
                        En[0:64, r0 + 1:r0 + 9, 1:1 + W])

                def dec_sb(t, k):
                    Bc, Bn = Bs[t % 2], Bs[(t + 1) % 2]
                    r0, c0 = 8 * k, 1024 * k
                    pA = pp.tile([128, 1024], F32, tag="pA", bufs=2)
                    pB = pp.tile([128, 1024], F32, tag="pB", bufs=2)
                    for ch, ps in ((0, pA), (1, pB)):
                        cw = slice(128 * ch, 128 * ch + 128)
                        for hf in range(2):
                            rh = r0 + 4 * hf
                            out = ps[:, 512 * hf:512 * hf + 512]
                            for s in range(9):
                                ky, kx = s // 3, s % 3
                                nc.tensor.matmul(
                                    out, decw[:, s, cw],
                                    Bc[:, rh + ky:rh + ky + 4, kx:kx + 128],
                                    start=(s == 0), stop=(s == 8))
                    t_if = tp.tile([128, 1024], F32, tag="t_if", bufs=2, name="t_if")
                    t_u = tp.tile([128, 1024], BF16, tag="t_u", bufs=2, name="t_u")
                    v1 = tp.tile([128, 1024], BF16, tag="t_v", bufs=3, name="v1")
                    v2 = tp.tile([128, 1024], BF16, tag="t_v", bufs=3, name="v2")
                    th = tp.tile([128, 1024], BF16, tag="t_v", bufs=3, name="th")
                    cblk = CC[0:64, c0:c0 + 1024]
                    # dec chunk A is [f|i] (host-permuted); tanh/relu ACT
                    # outputs criss-cross partitions so DVE input bases match
                    nc.scalar.activation(t_if[:], pA[:], AF.Relu,
                                         bias=aps[:, 2:3], scale=0.2)
                    nc.scalar.activation(t_u[64:128, :], pB[0:64, :], AF.Tanh,
                                         bias=aps[0:64, 3:4], scale=1.0)
                    nc.scalar.activation(t_u[0:64, :], pB[64:128, :], AF.Relu,
                                         bias=aps[64:128, 3:4], scale=0.2)
                    nc.vector.scalar_tensor_tensor(
                        v1[0:64, :], t_if[64:128, :], 1.0, t_u[64:128, :],
                        ALU.min, ALU.mult)
                    nc.vector.scalar_tensor_tensor(
                        v2[0:64, :], t_if[0:64, :], 1.0, cblk,
                        ALU.min, ALU.mult)
                    nc.vector.tensor_tensor(cblk, v1[0:64, :], v2[0:64, :],
                                            ALU.add)
                    nc.scalar.activation(th[0:64, :], cblk, AF.Tanh)
                    # h_dec -> B_next bottom (strided image write)
                    nc.vector.scalar_tensor_tensor(
                        Bn[64:128, r0 + 1:r0 + 9, 1:1 + W],
                        t_u[0:64, :].rearrange("p (a b) -> p a b", a=8), 1.0,
                        th[0:64, :].rearrange("p (a b) -> p a b", a=8),
                        ALU.min, ALU.mult)
                    # stage h_dec to DRAM for conv3d: full 130-wide rows
                    # (contiguous; Bn ring cols give dstage cols 0/129 = 0,
                    # corrected by colfix in the conv3d combine)
                    nc.gpsimd.dma_start(
                        dstage[t, :, r0 + 1:r0 + 9, :],
                        Bn[64:128, r0 + 1:r0 + 9, :])

                for t in range(T):
                    # dstage[t] top/bottom pad rows = -t_d/s_d (contiguous,
                    # cheap); col rings stay 0 and are fixed via colfix
                    nc.scalar.dma_start(dstage[t, :, 0, :], zbD[:])
                    nc.scalar.dma_start(dstage[t, :, PW - 1, :], zbD[:])
                    # decoder lags the encoder by 2 superblocks so the
                    # h_enc SBUF->SBUF DMA into B has ~17us to land
                    for k in range(NSB):
                        enc_sb(t, k)
                        if k >= 3:
                            dec_sb(t, k - 3)
                    for k in (NSB - 3, NSB - 2, NSB - 1):
                        dec_sb(t, k)

            # ---------------- conv3d + sigmoid ----------------
            # ky folded into M (90 = 3ky x 30 outputs): 15 matmuls per
            # 4-row window; ky-shifted partial sums combined on DVE.
            with tc.tile_pool(name="cp", bufs=1) as cp, \
                 tc.tile_pool(name="cpp", bufs=1, space=bass.MemorySpace.PSUM) as cpp:
                D = [cp.tile([128, PW, PW], BF16, name=f"D{q}") for q in range(5)]
                # load row slices of all frames top-first so early windows
                # can start while the rest streams in
                for rr in (slice(0, 34), slice(34, 66), slice(66, 98),
                           slice(98, PW)):
                    for q in range(5):
                        for j in range(2):
                            eng = (nc.sync, nc.gpsimd, nc.scalar)[(2 * q + j) % 3]
                            eng.dma_start(D[q][64 * j:64 * j + 64, rr, :],
                                          dstage[2 * q + j, :, rr, :])

                def evict(w):
                    # window w streams D_pad rows w..w+nw-1
                    nw = 2 if w == H else 4
                    py = cpp.tile([96, 512], F32, tag="py", bufs=3, name="py")
                    for q in range(5):
                        for kx in range(3):
                            i = 3 * q + kx
                            nc.tensor.matmul(
                                py[:, 0:128 * nw], w3t[:, i, :],
                                D[q][:, w:w + nw, kx:kx + 128],
                                start=(i == 0), stop=(i == 14))
                    shA = cp.tile([32, 512], F32, tag="shA", bufs=2, name="shA")
                    shB = cp.tile([32, 512], F32, tag="shB", bufs=2, name="shB")
                    shC = cp.tile([32, 512], F32, tag="shC", bufs=2, name="shC")
                    nc.scalar.activation(shA[:, 0:128 * nw], py[0:32, 0:128 * nw],
                                         AF.Copy)
                    nc.scalar.activation(shB[:, 0:128 * nw], py[32:64, 0:128 * nw],
                                         AF.Copy)
                    nc.scalar.activation(shC[:, 0:128 * nw], py[64:96, 0:128 * nw],
                                         AF.Copy)
                    return shA, shB, shC

                prev = evict(0)
                for k in range(NSB * 2):
                    y0 = 4 * k
                    cur, prev = prev, evict(y0 + 4)
                    cA, cB, cC = cur
                    nA, nB, nC = prev
                    u = cp.tile([30, 4, 128], F32, tag="u", bufs=2, name="u")
                    v = cp.tile([30, 4, 128], F32, tag="v", bufs=2, name="v")
                    cA = cA[0:30, :].rearrange("p (a b) -> p a b", b=128)
                    cB = cB[0:30, :].rearrange("p (a b) -> p a b", b=128)
                    cC = cC[0:30, :].rearrange("p (a b) -> p a b", b=128)
                    nB = nB[0:30, :].rearrange("p (a b) -> p a b", b=128)
                    nC = nC[0:30, :].rearrange("p (a b) -> p a b", b=128)
                    nc.vector.tensor_tensor(u[:, 0:3, :], cA[:, 0:3, :],
                                            cB[:, 1:4, :], ALU.add)
                    nc.vector.tensor_tensor(u[:, 3:4, :], cA[:, 3:4, :],
                                            nB[:, 0:1, :], ALU.add)
                    nc.vector.tensor_tensor(v[:, 0:2, :], u[:, 0:2, :],
                                            cC[:, 2:4, :], ALU.add)
                    nc.vector.tensor_tensor(v[:, 2:4, :], u[:, 2:4, :],
                                            nC[:, 0:2, :], ALU.add)
                    # col-ring correction: dstage cols 0/129 hold 0 instead
                    # of -t_d/s_d; add the baked per-(frame,ch) constants
                    for col, c0f in ((0, 0), (127, 3)):
                        segs = [(slice(0, 4), c0f + 1)]
                        if k == 0:
                            segs = [(slice(0, 1), c0f), (slice(1, 4), c0f + 1)]
                        elif k == 2 * NSB - 1:
                            segs = [(slice(0, 3), c0f + 1), (slice(3, 4), c0f + 2)]
                        for rs, ci in segs:
                            nc.vector.tensor_scalar(
                                v[:, rs, col:col + 1], v[:, rs, col:col + 1],
                                colfix[:, ci:ci + 1], None, ALU.add)
                    ty = cp.tile([30, 512], F32, tag="ty", bufs=2, name="ty")
                    nc.scalar.activation(ty[:], v[:].rearrange("p a b -> p (a b)"),
                                         AF.Sigmoid, bias=bout[:], scale=1.0)
                    nc.scalar.dma_start(
                        y_d[:, y0:y0 + 4, :],
                        ty[:].rearrange("p (a b) -> p a b", a=4))

    if do_split:
        split_multi_waits(nc)
    nc.finalize()
    return nc


def _prep(inputs):
    x = np.asarray(inputs["x"], np.float32)
    xpad = np.zeros((8, T, PW, PW, C), np.float32)
    xpad[:, :, 1:1 + H, 1:1 + W, :] = x
    xim = np.empty((8, T, 27, NPIX), BFP)
    for ky in range(3):
        for kx in range(3):
            s = ky * 3 + kx
            v = xpad[:, :, ky:ky + H, kx:kx + W, :]
            xim[:, :, s * 3:s * 3 + 3, :] = (
                v.transpose(0, 1, 4, 2, 3).reshape(8, T, 3, NPIX).astype(BFP))

    enc_Wh = np.asarray(inputs["enc_Wh"], np.float32)
    enc_Wx = np.asarray(inputs["enc_Wx"], np.float32)
    dec_Wx = np.asarray(inputs["dec_Wx"], np.float32)
    dec_Wh = np.asarray(inputs["dec_Wh"], np.float32)
    out_W = np.asarray(inputs["out_W"], np.float32)
    enc_b = np.asarray(inputs["enc_b"], np.float32)
    dec_b = np.asarray(inputs["dec_b"], np.float32)

    # BN affine folds
    s_e = np.asarray(inputs["enc_gamma"], np.float32) / np.sqrt(
        np.asarray(inputs["enc_var"], np.float32) + BN_EPS)
    t_e = np.asarray(inputs["enc_beta"], np.float32) - np.asarray(
        inputs["enc_mean"], np.float32) * s_e
    s_d = np.asarray(inputs["dec_gamma"], np.float32) / np.sqrt(
        np.asarray(inputs["dec_var"], np.float32) + BN_EPS)
    t_d = np.asarray(inputs["dec_beta"], np.float32) - np.asarray(
        inputs["dec_mean"], np.float32) * s_d

    # encoder hidden-conv weights: paired taps (ky,0)+(ky,1), singles (ky,2)
    encp = np.zeros((128, 3, 256), np.float32)
    encs = np.zeros((128, 3, 256), np.float32)
    for ky in range(3):
        encp[0:64, ky, :] = enc_Wh[ky, 0]
        encp[64:128, ky, :] = enc_Wh[ky, 1]
        encs[0:64, ky, :] = enc_Wh[ky, 2]
    encx = np.zeros((128, 256), np.float32)
    encx[0:27, :] = enc_Wx.reshape(27, 256)

    # decoder weights: rows 0:64 dec_Wx scaled by s_e (enc-BN fold),
    # rows 64:128 dec_Wh; gate chunk A permuted to [f|i] so DVE two-input
    # ops get matching base partitions
    perm = np.concatenate([np.arange(64, 128), np.arange(0, 64),
                           np.arange(128, 256)])
    decw = np.zeros((128, 9, 256), np.float32)
    for s in range(9):
        ky, kx = s // 3, s % 3
        decw[0:64, s, :] = (dec_Wx[ky, kx] * s_e[:, None])[:, perm]
        decw[64:128, s, :] = dec_Wh[ky, kx][:, perm]
    # enc-BN shift folded into decoder bias (interior taps; border taps
    # cancelled by the -t_e/s_e ring in B's top half)
    dec_b_f = dec_b + np.einsum("ykcg,c->g", dec_Wx.reshape(3, 3, 64, 256), t_e)

    # conv3d weights scaled by s_d (dec-BN fold); ky folded into M:
    # col 32*ky + (3t+c), 32-row groups for partition-base alignment
    out_W_s = out_W * s_d[None, None, None, :, None]
    w3 = np.zeros((15, 128, 96), np.float32)
    for q in range(5):
        for j in range(2):
            f = 2 * q + j
            for t in range(max(0, f - 1), min(T - 1, f + 1) + 1):
                dt = f - t + 1
                for ky in range(3):
                    for kx in range(3):
                        w3[q * 3 + kx, 64 * j:64 * j + 64,
                           32 * ky + 3 * t:32 * ky + 3 * t + 3] = \
                            out_W_s[dt, ky, kx]
    w3 = w3.transpose(1, 0, 2)  # [128, 15, 96]

    # conv3d bias: out_b + dec-BN shift over valid temporal taps
    out_b = np.asarray(inputs["out_b"], np.float32)
    bout = np.zeros((30,), np.float32)
    for t in range(T):
        acc = out_b.copy()
        for dt in range(3):
            g = t + dt - 1
            if 0 <= g < T:
                acc = acc + np.einsum("ykcg,c->g", out_W[dt], t_d)
        bout[3 * t:3 * t + 3] = acc
    bout = bout.reshape(30, 1)

    aps = np.zeros((128, 4), np.float32)
    aps[0:64, 0] = 0.2 * enc_b[0:64] + 0.5        # enc i
    aps[64:128, 0] = 0.2 * enc_b[64:128] + 0.5    # enc f
    aps[0:64, 1] = enc_b[128:192]                 # enc c~ (tanh bias)
    aps[64:128, 1] = 0.2 * enc_b[192:256] + 0.5   # enc o
    aps[0:64, 2] = 0.2 * dec_b_f[64:128] + 0.5    # dec f (chunk A is [f|i])
    aps[64:128, 2] = 0.2 * dec_b_f[0:64] + 0.5    # dec i
    aps[0:64, 3] = dec_b_f[128:192]               # dec c~
    aps[64:128, 3] = 0.2 * dec_b_f[192:256] + 0.5 # dec o

    # col-ring correction constants: output (t,y,x=0/127) misses the
    # -t_d/s_d ring under tap kx=0/2 for ky reaching interior rows
    colfix = np.zeros((30, 6), np.float32)
    for t in range(T):
        for side, kx in ((0, 0), (3, 2)):
            for ci, kys in ((0, (1, 2)), (1, (0, 1, 2)), (2, (0, 1))):
                acc = np.zeros(3, np.float32)
                for dt in range(3):
                    g = t + dt - 1
                    if 0 <= g < T:
                        for ky in kys:
                            acc -= np.einsum("cg,c->g", out_W[dt, ky, kx], t_d)
                colfix[3 * t:3 * t + 3, side + ci] = acc

    ringb = np.zeros((128, PW), np.float32)
    ringb[0:64, :] = (-t_e / s_e)[:, None]
    ringd = np.tile((-t_d / s_d)[:, None], (1, PW))

    shared = {
        "encp": encp.reshape(128, 3 * 256).astype(BFP),
        "encs": encs.reshape(128, 3 * 256).astype(BFP),
        "encx": encx.astype(BFP),
        "decw": decw.reshape(128, 9 * 256).astype(BFP),
        "w3": w3.reshape(128, 15 * 96).astype(BFP),
        "aps": aps,
        "bout": bout,
        "ringb": ringb.astype(BFP),
        "ringd": ringd.astype(BFP),
        "colfix": colfix,
    }
    return [dict(shared, xim=np.ascontiguousarray(xim[c])) for c in range(8)]


_CACHE = {}


def kernel(**inputs):
    if "nc" not in _CACHE:
        _CACHE["nc"] = _build()
    nc = _CACHE["nc"]
    in_maps = _prep(inputs)
    from concourse.bass_utils import run_bass_kernel_spmd
    res = run_bass_kernel_spmd(nc, in_maps, core_ids=list(range(8)))
    kernel.last_exec_ns = res.exec_time_ns
    kernel.last_res = res
    y = np.stack([
        np.asarray(res.results[c]["y"], np.float32)
        .reshape(T, 3, H, W).transpose(0, 2, 3, 1)
        for c in range(8)
    ])
    return y
